# revision 13
# baseline (speedup 1.0000x reference)
"""DistMult+KBLN scoring kernel for 8 Trainium2 NeuronCores.

Math (eval mode, per reference):
    e1 = E[e1_idx]; r = R[r_idx]                       [B, D]
    score_l[b,e] = sum_d (e1*r)[b,d] * E[e,d]
    score_n[b,e] = sum_l nf[r_idx][b,l] * exp(-((n_h[b,l]-num_lit[e,l]-c[l])^2/var[l]))
    out = sigmoid(score_l + score_n)                   [B, E]

Key idea: the RBF factor phi(a - m) is a smooth 1-D Gaussian in the
num_lit value m, so expand it per-literal in a cubic B-spline basis on a
knot grid over m (spacing H*sigma_l):

    phi(a[b,l] - m[e,l]) ~= sum_j c_j(a[b,l]) * B3((m[e,l]-xi_j)/h_l)

Then score_n[b,e] = sum_{l,j} (w[b,l]*c_{l,j}) * Bmat[(l,j), e] is ONE
matmul with contraction dim K ~= 1650 (score_l's 200 E-rows are appended
to the same contraction).  The basis matrix is stored in FP8 (E3M4) with
error-diffusion rounding along the 4-tap spline window (the taps' errors
cancel against the smooth spline coefficients), and the per-batch
coefficients are least-squares refit against the *quantized* basis
sampled at actual entity literal values.  The stationary lhs stays FP16
(the PE supports fp16-stationary x fp8-moving matmuls).  The device
kernel is a pure streaming matmul: ~8 MB of fp8 basis per core streamed
from HBM at full DMA bandwidth, accumulated in PSUM (10 sub-accumulators
as halves of 5 banks), sigmoid on the way out, fp16 store.

Sharding: entity axis split row-wise across 8 cores (5000 entities
each); host concatenates. No collectives.
"""
import sys

if "/opt/trn_rl_repo" not in sys.path:
    sys.path.insert(0, "/opt/trn_rl_repo")

import ml_dtypes
import numpy as np

import concourse.bass as bass
import concourse.mybir as mybir
import concourse.tile as _tile
from concourse import tile
from concourse.bass_utils import run_bass_kernel_spmd
from concourse.vector_clock import ScopedClock

B = 64
NUM_ENT = 40000
NUM_REL = 1345
DIM = 200
N_LIT = 100
NCORES = 8
ESH = NUM_ENT // NCORES  # 5000 entities per core

SUBW = 500   # matmul free dim (one PSUM bank holds 512 fp32)
H = 0.61     # B-spline knot spacing in units of sigma_l
GUARD = 4    # extra knots beyond 1/h (cubic overhang)
S8 = 16.0    # fp8 scale: basis values stored as fp8(S8*B3), lhs carries 1/S8
FP8 = ml_dtypes.float8_e3m4

f32 = mybir.dt.float32
f16 = mybir.dt.float16
f8 = mybir.dt.float8e3
AF = mybir.ActivationFunctionType


def _drain_and_barrier_split(self, tick_clock, wait_clock):
    # This walrus build rejects >1 sync-wait per instruction; the tail Drain
    # normally carries one wait per active processor. Collect them on a probe
    # NOP instead (split later by _split_multi_waits) and emit a clean drain.
    nc = self.nc
    probe = nc.sync.nop(nofuse=True, hint="tail_wait_probe")
    wait_clock.add_sem_waits(probe.ins, ScopedClock({None: tick_clock.global_clock}))
    nc.sync.drain()
    nc.all_engine_barrier()
    assert self.sems is not None
    popped = nc._tile_sem_poison_stack.pop()
    assert popped is self._sem_poison
    nc.clear_and_free_semaphores(list(self.sems.allocated().values()))
    nc.all_engine_barrier()


_tile.TileContext._drain_and_barrier = _drain_and_barrier_split


def _split_multi_waits(nc: bass.Bass) -> int:
    """Hoist all-but-one sync wait from every instruction onto standalone
    single-wait EventSemaphore instructions inserted just before it (same
    engine, same block). Needed because this walrus build errors with
    "Too many sync wait commands" on instructions carrying >1 wait."""
    n_split = 0
    for bb in nc.m.functions[0].blocks:
        new_insts = []
        for inst in bb.instructions:
            waits = list(inst.sync_info.on_wait) if inst.sync_info else []
            if len(waits) > 1:
                for sw in waits[:-1]:
                    ev = mybir.InstEventSemaphore(
                        name=nc.get_next_instruction_name(),
                        engine=inst.engine,
                        ins=[],
                        outs=[],
                        sync_info=mybir.SyncInfo(on_wait=[sw], on_update=[]),
                    )
                    nc.register_instruction(ev)
                    new_insts.append(ev)
                    n_split += 1
                inst.sync_info.on_wait = waits[-1:]
            new_insts.append(inst)
        bb.instructions[:] = new_insts
    return n_split


def _xfer_plan(kc: int):
    """DMA transfer plan over chunks: ('p', ck) = pair (ck, ck+1) of full
    128-row chunks, ('s', ck) = one leftover full single, ('l', kc-1) =
    partial last chunk. Pairs give 10KB descriptor lines (the 16 DMA
    engines are latency-bound ~376ns/descriptor below ~10KB)."""
    n_full = kc - 1
    plan = []
    ck = 0
    while ck + 1 < n_full:
        plan.append(("p", ck))
        ck += 2
    if ck < n_full:
        plan.append(("s", ck))
    plan.append(("l", kc - 1))
    return plan


def build_nc(kc: int, k_last: int) -> bass.Bass:
    """kc = number of contraction chunks; last chunk has k_last (<=128) rows.

    All chunk tiles are resident in SBUF (no buffer reuse), and all input
    DMAs are issued upfront so the 16 DMA engines never starve."""
    nc = bass.Bass()
    plan = _xfer_plan(kc)
    n_single = sum(1 for t, _ in plan if t == "s")
    n_pair = sum(1 for t, _ in plan if t == "p")

    if n_single:
        Bms_d = nc.dram_tensor("Bms", [n_single, 128, ESH], f8,
                               kind="ExternalInput")
    if n_pair:
        Bmp_d = nc.dram_tensor("Bmp", [n_pair, 128, 2 * ESH], f8,
                               kind="ExternalInput")
    Bml_d = nc.dram_tensor("Bml", [k_last, ESH], f8, kind="ExternalInput")
    lhs_d = nc.dram_tensor("lhs", [128, kc * B], f16, kind="ExternalInput")
    # out[half, b, x] = sigmoid score for entity half*HALF+x -> 5000B dram
    # lines per (half, b), so the stores use big contiguous descriptors
    out_d = nc.dram_tensor("out", [2, B, ESH // 2], f16, kind="ExternalOutput")

    HALF = ESH // 2   # 2500
    NS = ESH // SUBW  # 10 sub-accumulators; 2 per PSUM bank (split partitions)

    with tile.TileContext(nc) as tc:
        with (
            tc.tile_pool(name="const", bufs=1) as cpool,
            tc.tile_pool(name="ps", bufs=1, space=bass.MemorySpace.PSUM) as pspool,
            tc.tile_pool(name="acc", bufs=1) as accpool,
        ):
            lhs_sb = cpool.tile([128, kc * B], f16, tag="lhs")
            warm = cpool.tile([128, 1], f32, tag="warm")
            out2 = accpool.tile([128, HALF], f16, tag="outsb")

            # load the Sigmoid act table early (it's the only table this
            # kernel uses, so the tail pays no table switch)
            nc.scalar.activation(warm[:], warm[:], AF.Sigmoid)

            # All DMAs upfront on gpsimd = SWDGE (descriptors spread across
            # all 16 DMA engines). The lhs has tiny 2*kc*B-byte lines
            # (latency-bound descriptors), so it is sliced per transfer
            # group and interleaved into the pair flow: each slice rides
            # just ahead of the chunk pair that needs it. Pair 0 is split
            # in column halves so the PE can start ~2us earlier.
            def lhs_slice(c0, c1):
                nc.gpsimd.dma_start(
                    lhs_sb[:, c0 * B : c1 * B], lhs_d[:, c0 * B : c1 * B]
                )

            chunk_src = {}   # ck -> (tile, col_off, rows)
            si = pi = 0
            for t, ck in plan:
                if t == "p":
                    tl = cpool.tile([128, 2 * ESH], f8, tag=f"btp{pi}")
                    lhs_slice(ck, ck + 2)
                    if pi == 0:
                        nc.gpsimd.dma_start(
                            tl[:, 0 : ESH // 2], Bmp_d[pi, :, 0 : ESH // 2]
                        )
                        nc.gpsimd.dma_start(
                            tl[:, ESH // 2 :], Bmp_d[pi, :, ESH // 2 :]
                        )
                    else:
                        nc.gpsimd.dma_start(tl[:], Bmp_d[pi, :, :])
                    chunk_src[ck] = (tl, 0, 128)
                    chunk_src[ck + 1] = (tl, ESH, 128)
                    pi += 1
                elif t == "s":
                    tl = cpool.tile([128, ESH], f8, tag=f"bts{si}")
                    lhs_slice(ck, ck + 1)
                    nc.gpsimd.dma_start(tl[:], Bms_d[si, :, :])
                    chunk_src[ck] = (tl, 0, 128)
                    si += 1
                else:
                    tl = cpool.tile([128, ESH], f8, tag="btl")
                    lhs_slice(ck, ck + 1)
                    nc.gpsimd.dma_start(tl[0:k_last, :], Bml_d[:])
                    chunk_src[ck] = (tl, 0, k_last)

            ps = [
                pspool.tile([128, SUBW], f32, tag=f"ps{s}", name=f"ps_{s}")
                for s in range(NS // 2)
            ]

            def acc_mm(ck, s, start, stop):
                t, off, rows = chunk_src[ck]
                bank, half = s % 5, s // 5
                nc.tensor.matmul(
                    ps[bank][half * B : (half + 1) * B, :],
                    lhs_sb[0:rows, ck * B : (ck + 1) * B],
                    t[0:rows, off + s * SUBW : off + (s + 1) * SUBW],
                    start=start, stop=stop, tile_position=(0, half * B),
                )

            # chunks 0..kc-3: plain order
            for ck in range(kc - 2):
                for s in range(NS):
                    acc_mm(ck, s, start=(ck == 0), stop=False)
            # final two chunks interleaved bank-major so each PSUM bank's
            # accumulation STOPS early and its sigmoid pipelines under the
            # remaining matmuls (sigmoid chain ends ~0.7us after last matmul)
            for s in (0, 5, 1, 6, 2, 7, 3, 8, 4, 9):
                acc_mm(kc - 2, s, start=False, stop=False)
                acc_mm(kc - 1, s, start=False, stop=True)

            # tail: per PSUM bank, sigmoid straight from PSUM into fp16
            # (rows 0:64 = entity half [0,2500), rows 64:128 = [2500,5000));
            # thanks to the bank-major interleave above the sigmoid chain
            # pipelines under the final matmuls; then ONE full-width store
            # (5000B lines; a store burns >=128 descriptors regardless of
            # width, so one wide store beats several narrow ones)
            for bank in range(NS // 2):
                c0 = bank * SUBW
                nc.scalar.activation(
                    out2[:, c0 : c0 + SUBW], ps[bank][:], AF.Sigmoid,
                )
            nc.gpsimd.dma_start(out_d[:, :, :], out2[:, :])

    _split_multi_waits(nc)
    return nc


def _bspline3(t):
    at = np.abs(t)
    r = np.zeros_like(at)
    m1 = at < 1
    r[m1] = (4 - 6 * at[m1] ** 2 + 3 * at[m1] ** 3) / 6
    m2 = (at >= 1) & (at < 2)
    r[m2] = ((2 - at[m2]) ** 3) / 6
    return r


def _rtn8(x):
    """Round f64 -> fp8e3 representable, returned as f64 (same scale)."""
    return np.asarray(x, dtype=np.float32).astype(FP8).astype(np.float64)


def make_host_data(e1_idx, r_idx, E_weight, R_weight, num_lit, c, var, nf_weights):
    """Build the fp8 basis matrix [K, NUM_ENT] and packed fp16 lhs."""
    e1_idx = np.asarray(e1_idx).astype(np.int64)
    r_idx = np.asarray(r_idx).astype(np.int64)
    E_weight = np.asarray(E_weight, dtype=np.float64)
    R_weight = np.asarray(R_weight, dtype=np.float64)
    num_lit = np.asarray(num_lit, dtype=np.float64)
    c = np.asarray(c, dtype=np.float64)
    var = np.asarray(var, dtype=np.float64)
    nf = np.asarray(nf_weights, dtype=np.float64)

    sig = np.sqrt(var)                  # [L]
    a_ctr = num_lit[e1_idx] - c         # [B, L] Gaussian centers, z units
    w = nf[r_idx]                       # [B, L]

    Js = np.array([int(np.ceil(1.0 / (H * s))) + GUARD for s in sig])
    offs = np.concatenate([[0], np.cumsum(Js)[:-1]])
    K_phi = int(Js.sum())
    K = K_phi + DIM
    kc = (K + 127) // 128
    k_last = K - (kc - 1) * 128

    Bmat = np.zeros((K, NUM_ENT), dtype=np.float64)
    lhsT = np.zeros((K, B), dtype=np.float16)
    eidx = np.arange(NUM_ENT)
    sidx = np.arange(0, NUM_ENT, NUM_ENT // 4000)   # refit subsample
    for l in range(N_LIT):
        hz = H * sig[l]
        J = int(Js[l])
        off = int(offs[l])
        xi0 = -2 * hz
        t = (num_lit[:, l] - xi0) / hz
        j0 = np.clip(np.floor(t).astype(np.int64), 1, J - 3)
        # 4-tap window of B-spline values, quantized to fp8 (at scale S8)
        # with error diffusion along the taps: the taps' coefficients vary
        # smoothly, so pushing each tap's rounding error into the next tap
        # cancels the bulk of the quantization noise in the weighted sum.
        acc = np.zeros(NUM_ENT)
        for k in range(4):
            v = _bspline3(t - (j0 - 1 + k)) * S8
            q = _rtn8(v + acc)
            acc = v + acc - q
            Bmat[off + (j0 - 1 + k), eidx] = q
        # refit: per-batch LSQ of the true Gaussian against the *quantized*
        # basis at a subsample of actual entity literal values (this also
        # absorbs the 1/S8 scale into the coefficients)
        Bs = Bmat[off : off + J][:, sidx]
        G = Bs @ Bs.T + 1e-8 * np.eye(J)
        phi = np.exp(
            -(((a_ctr[:, l][:, None] - num_lit[sidx, l][None, :]) / sig[l]) ** 2)
        )
        C = np.linalg.solve(G, Bs @ phi.T).T            # [B, J]
        lhsT[off : off + J, :] = (C * w[:, l][:, None]).T.astype(np.float16)

    # append the DistMult rows: score_l = (e1*r) @ E^T
    x = E_weight[e1_idx] * R_weight[r_idx]              # [B, D]
    Bmat[K_phi : K_phi + DIM, :] = _rtn8(E_weight.T * S8)
    lhsT[K_phi : K_phi + DIM, :] = (x.T / S8).astype(np.float16)

    Bmat8 = Bmat.astype(np.float32).astype(FP8)         # exact (pre-rounded)

    # pack lhs chunks (zero-pad K -> kc*128): lhs_pack[p, ck*B+b] = lhsT[ck*128+p, b]
    lhs_pad = np.zeros((kc * 128, B), dtype=np.float16)
    lhs_pad[:K] = lhsT
    lhs_pack = np.ascontiguousarray(
        lhs_pad.reshape(kc, 128, B).transpose(1, 0, 2).reshape(128, kc * B)
    )
    return kc, k_last, Bmat8, lhs_pack


def make_in_maps_from(kc, k_last, Bmat8, lhs_pack):
    plan = _xfer_plan(kc)
    n_single = sum(1 for t, _ in plan if t == "s")
    n_pair = sum(1 for t, _ in plan if t == "p")
    in_maps = []
    for core in range(NCORES):
        sl = slice(core * ESH, (core + 1) * ESH)
        Bc = Bmat8[:, sl]
        singles = np.empty((n_single, 128, ESH), dtype=FP8)
        pairs = np.empty((n_pair, 128, 2 * ESH), dtype=FP8)
        m = {"lhs": lhs_pack}
        if n_single:
            m["Bms"] = singles
        si = pi = 0
        for t, ck in plan:
            if t == "s":
                singles[si] = Bc[ck * 128 : (ck + 1) * 128]
                si += 1
            elif t == "p":
                pairs[pi, :, :ESH] = Bc[ck * 128 : (ck + 1) * 128]
                pairs[pi, :, ESH:] = Bc[(ck + 1) * 128 : (ck + 2) * 128]
                pi += 1
            else:
                m["Bml"] = np.ascontiguousarray(Bc[ck * 128 :])
        if n_pair:
            m["Bmp"] = pairs
        in_maps.append(m)
    return in_maps


def make_in_maps(**inputs):
    kc, k_last, Bmat8, lhs_pack = make_host_data(**inputs)
    return make_in_maps_from(kc, k_last, Bmat8, lhs_pack)


_NC_CACHE = {}


def kernel(**inputs) -> np.ndarray:
    kc, k_last, Bmat8, lhs_pack = make_host_data(**inputs)
    if (kc, k_last) not in _NC_CACHE:
        _NC_CACHE[(kc, k_last)] = build_nc(kc, k_last)
    nc = _NC_CACHE[(kc, k_last)]
    in_maps = make_in_maps_from(kc, k_last, Bmat8, lhs_pack)
    res = run_bass_kernel_spmd(nc, in_maps, list(range(NCORES)))
    # per-core out is [2, B, ESH//2] (entity halves); reassemble to [B, ESH]
    out = np.concatenate(
        [
            np.concatenate(
                [np.asarray(res.results[i]["out"][h]) for h in range(2)], axis=1
            )
            for i in range(NCORES)
        ],
        axis=1,
    )
    return out.astype(np.float32)


# revision 14
# speedup vs baseline: 1.0321x; 1.0321x over previous
"""DistMult+KBLN scoring kernel for 8 Trainium2 NeuronCores.

Math (eval mode, per reference):
    e1 = E[e1_idx]; r = R[r_idx]                       [B, D]
    score_l[b,e] = sum_d (e1*r)[b,d] * E[e,d]
    score_n[b,e] = sum_l nf[r_idx][b,l] * exp(-((n_h[b,l]-num_lit[e,l]-c[l])^2/var[l]))
    out = sigmoid(score_l + score_n)                   [B, E]

Key idea: the RBF factor phi(a - m) is a smooth 1-D Gaussian in the
num_lit value m, so expand it per-literal in a cubic B-spline basis on a
knot grid over m (spacing H*sigma_l):

    phi(a[b,l] - m[e,l]) ~= sum_j c_j(a[b,l]) * B3((m[e,l]-xi_j)/h_l)

Then score_n[b,e] = sum_{l,j} (w[b,l]*c_{l,j}) * Bmat[(l,j), e] is ONE
matmul with contraction dim K ~= 1650 (score_l's 200 E-rows are appended
to the same contraction).  The basis matrix is stored in FP8 (E3M4) with
error-diffusion rounding along the 4-tap spline window (the taps' errors
cancel against the smooth spline coefficients), and the per-batch
coefficients are least-squares refit against the *quantized* basis
sampled at actual entity literal values.  The stationary lhs stays FP16
(the PE supports fp16-stationary x fp8-moving matmuls).  The device
kernel is a pure streaming matmul: ~8 MB of fp8 basis per core streamed
from HBM at full DMA bandwidth, accumulated in PSUM (10 sub-accumulators
as halves of 5 banks), sigmoid on the way out, fp16 store.

Sharding: entity axis split row-wise across 8 cores (5000 entities
each); host concatenates. No collectives.
"""
import sys

if "/opt/trn_rl_repo" not in sys.path:
    sys.path.insert(0, "/opt/trn_rl_repo")

import ml_dtypes
import numpy as np

import concourse.bass as bass
import concourse.mybir as mybir
import concourse.tile as _tile
from concourse import tile
from concourse.bass_utils import run_bass_kernel_spmd
from concourse.vector_clock import ScopedClock

B = 64
NUM_ENT = 40000
NUM_REL = 1345
DIM = 200
N_LIT = 100
NCORES = 8
ESH = NUM_ENT // NCORES  # 5000 entities per core

SUBW = 500   # matmul free dim (one PSUM bank holds 512 fp32)
H = 0.61     # B-spline knot spacing in units of sigma_l
GUARD = 4    # extra knots beyond 1/h (cubic overhang)
S8 = 16.0    # fp8 scale: basis values stored as fp8(S8*B3), lhs carries 1/S8
FP8 = ml_dtypes.float8_e3m4

f32 = mybir.dt.float32
f16 = mybir.dt.float16
f8 = mybir.dt.float8e3
AF = mybir.ActivationFunctionType


def _drain_and_barrier_split(self, tick_clock, wait_clock):
    # This walrus build rejects >1 sync-wait per instruction; the tail Drain
    # normally carries one wait per active processor. Collect them on a probe
    # NOP instead (split later by _split_multi_waits) and emit a clean drain.
    nc = self.nc
    probe = nc.sync.nop(nofuse=True, hint="tail_wait_probe")
    wait_clock.add_sem_waits(probe.ins, ScopedClock({None: tick_clock.global_clock}))
    nc.sync.drain()
    nc.all_engine_barrier()
    assert self.sems is not None
    popped = nc._tile_sem_poison_stack.pop()
    assert popped is self._sem_poison
    nc.clear_and_free_semaphores(list(self.sems.allocated().values()))
    nc.all_engine_barrier()


_tile.TileContext._drain_and_barrier = _drain_and_barrier_split


def _split_multi_waits(nc: bass.Bass) -> int:
    """Hoist all-but-one sync wait from every instruction onto standalone
    single-wait EventSemaphore instructions inserted just before it (same
    engine, same block). Needed because this walrus build errors with
    "Too many sync wait commands" on instructions carrying >1 wait."""
    n_split = 0
    for bb in nc.m.functions[0].blocks:
        new_insts = []
        for inst in bb.instructions:
            waits = list(inst.sync_info.on_wait) if inst.sync_info else []
            if len(waits) > 1:
                for sw in waits[:-1]:
                    ev = mybir.InstEventSemaphore(
                        name=nc.get_next_instruction_name(),
                        engine=inst.engine,
                        ins=[],
                        outs=[],
                        sync_info=mybir.SyncInfo(on_wait=[sw], on_update=[]),
                    )
                    nc.register_instruction(ev)
                    new_insts.append(ev)
                    n_split += 1
                inst.sync_info.on_wait = waits[-1:]
            new_insts.append(inst)
        bb.instructions[:] = new_insts
    return n_split


def _xfer_plan(kc: int):
    """DMA transfer plan over chunks: ('p', ck) = pair (ck, ck+1) of full
    128-row chunks, ('s', ck) = one leftover full single, ('l', kc-1) =
    partial last chunk. Pairs give 10KB descriptor lines (the 16 DMA
    engines are latency-bound ~376ns/descriptor below ~10KB)."""
    n_full = kc - 1
    plan = []
    ck = 0
    while ck + 1 < n_full:
        plan.append(("p", ck))
        ck += 2
    if ck < n_full:
        plan.append(("s", ck))
    plan.append(("l", kc - 1))
    return plan


def build_nc(kc: int, k_last: int) -> bass.Bass:
    """kc = number of contraction chunks; last chunk has k_last (<=128) rows.

    All chunk tiles are resident in SBUF (no buffer reuse), and all input
    DMAs are issued upfront so the 16 DMA engines never starve."""
    nc = bass.Bass()
    plan = _xfer_plan(kc)
    n_single = sum(1 for t, _ in plan if t == "s")
    n_pair = sum(1 for t, _ in plan if t == "p")

    if n_single:
        Bms_d = nc.dram_tensor("Bms", [n_single, 128, ESH], f8,
                               kind="ExternalInput")
    if n_pair:
        Bmp_d = nc.dram_tensor("Bmp", [n_pair, 128, 2 * ESH], f8,
                               kind="ExternalInput")
    Bml_d = nc.dram_tensor("Bml", [k_last, ESH], f8, kind="ExternalInput")
    lhs_d = nc.dram_tensor("lhs", [128, kc * B], f16, kind="ExternalInput")
    # out[half, b, x] = sigmoid score for entity half*HALF+x -> 5000B dram
    # lines per (half, b), so the stores use big contiguous descriptors
    out_d = nc.dram_tensor("out", [2, B, ESH // 2], f16, kind="ExternalOutput")

    HALF = ESH // 2   # 2500
    NS = ESH // SUBW  # 10 sub-accumulators; 2 per PSUM bank (split partitions)

    with tile.TileContext(nc) as tc:
        with (
            tc.tile_pool(name="const", bufs=1) as cpool,
            tc.tile_pool(name="ps", bufs=1, space=bass.MemorySpace.PSUM) as pspool,
            tc.tile_pool(name="acc", bufs=1) as accpool,
        ):
            lhs_sb = cpool.tile([128, kc * B], f16, tag="lhs")
            warm = cpool.tile([128, 1], f32, tag="warm")
            out2 = accpool.tile([128, HALF], f16, tag="outsb")

            # lhs through SWDGE (spread over 16 engines; it gates the
            # first matmul so it must land fast), issued before chunk 0
            nc.gpsimd.dma_start(lhs_sb[:], lhs_d[:])
            # load the Sigmoid act table early (it's the only table this
            # kernel uses, so the tail pays no table switch)
            nc.scalar.activation(warm[:], warm[:], AF.Sigmoid)

            # all chunk DMAs upfront on gpsimd = SWDGE (descriptors spread
            # across all 16 DMA engines)
            chunk_src = {}   # ck -> (tile, col_off, rows)
            si = pi = 0
            for t, ck in plan:
                if t == "p":
                    tl = cpool.tile([128, 2 * ESH], f8, tag=f"btp{pi}")
                    nc.gpsimd.dma_start(tl[:], Bmp_d[pi, :, :])
                    chunk_src[ck] = (tl, 0, 128)
                    chunk_src[ck + 1] = (tl, ESH, 128)
                    pi += 1
                elif t == "s":
                    tl = cpool.tile([128, ESH], f8, tag=f"bts{si}")
                    nc.gpsimd.dma_start(tl[:], Bms_d[si, :, :])
                    chunk_src[ck] = (tl, 0, 128)
                    si += 1
                else:
                    tl = cpool.tile([128, ESH], f8, tag="btl")
                    nc.gpsimd.dma_start(tl[0:k_last, :], Bml_d[:])
                    chunk_src[ck] = (tl, 0, k_last)

            ps = [
                pspool.tile([128, SUBW], f32, tag=f"ps{s}", name=f"ps_{s}")
                for s in range(NS // 2)
            ]

            def acc_mm(ck, s, start, stop):
                t, off, rows = chunk_src[ck]
                bank, half = s % 5, s // 5
                nc.tensor.matmul(
                    ps[bank][half * B : (half + 1) * B, :],
                    lhs_sb[0:rows, ck * B : (ck + 1) * B],
                    t[0:rows, off + s * SUBW : off + (s + 1) * SUBW],
                    start=start, stop=stop, tile_position=(0, half * B),
                )

            # chunks 0..kc-3: plain order
            for ck in range(kc - 2):
                for s in range(NS):
                    acc_mm(ck, s, start=(ck == 0), stop=False)
            # final two chunks interleaved bank-major so each PSUM bank's
            # accumulation STOPS early and its sigmoid pipelines under the
            # remaining matmuls (sigmoid chain ends ~0.7us after last matmul)
            for s in (0, 5, 1, 6, 2, 7, 3, 8, 4, 9):
                acc_mm(kc - 2, s, start=False, stop=False)
                acc_mm(kc - 1, s, start=False, stop=True)

            # tail: per PSUM bank, sigmoid straight from PSUM into fp16
            # (rows 0:64 = entity half [0,2500), rows 64:128 = [2500,5000));
            # thanks to the bank-major interleave above the sigmoid chain
            # pipelines under the final matmuls; then ONE full-width store
            # (5000B lines; a store burns >=128 descriptors regardless of
            # width, so one wide store beats several narrow ones)
            for bank in range(NS // 2):
                c0 = bank * SUBW
                nc.scalar.activation(
                    out2[:, c0 : c0 + SUBW], ps[bank][:], AF.Sigmoid,
                )
            nc.gpsimd.dma_start(out_d[:, :, :], out2[:, :])

    _split_multi_waits(nc)
    return nc


def _bspline3(t):
    at = np.abs(t)
    r = np.zeros_like(at)
    m1 = at < 1
    r[m1] = (4 - 6 * at[m1] ** 2 + 3 * at[m1] ** 3) / 6
    m2 = (at >= 1) & (at < 2)
    r[m2] = ((2 - at[m2]) ** 3) / 6
    return r


def _rtn8(x):
    """Round f64 -> fp8e3 representable, returned as f64 (same scale)."""
    return np.asarray(x, dtype=np.float32).astype(FP8).astype(np.float64)


def make_host_data(e1_idx, r_idx, E_weight, R_weight, num_lit, c, var, nf_weights):
    """Build the fp8 basis matrix [K, NUM_ENT] and packed fp16 lhs."""
    e1_idx = np.asarray(e1_idx).astype(np.int64)
    r_idx = np.asarray(r_idx).astype(np.int64)
    E_weight = np.asarray(E_weight, dtype=np.float64)
    R_weight = np.asarray(R_weight, dtype=np.float64)
    num_lit = np.asarray(num_lit, dtype=np.float64)
    c = np.asarray(c, dtype=np.float64)
    var = np.asarray(var, dtype=np.float64)
    nf = np.asarray(nf_weights, dtype=np.float64)

    sig = np.sqrt(var)                  # [L]
    a_ctr = num_lit[e1_idx] - c         # [B, L] Gaussian centers, z units
    w = nf[r_idx]                       # [B, L]

    Js = np.array([int(np.ceil(1.0 / (H * s))) + GUARD for s in sig])
    offs = np.concatenate([[0], np.cumsum(Js)[:-1]])
    K_phi = int(Js.sum())
    K = K_phi + DIM
    kc = (K + 127) // 128
    k_last = K - (kc - 1) * 128

    Bmat = np.zeros((K, NUM_ENT), dtype=np.float64)
    lhsT = np.zeros((K, B), dtype=np.float16)
    eidx = np.arange(NUM_ENT)
    sidx = np.arange(0, NUM_ENT, NUM_ENT // 4000)   # refit subsample
    for l in range(N_LIT):
        hz = H * sig[l]
        J = int(Js[l])
        off = int(offs[l])
        xi0 = -2 * hz
        t = (num_lit[:, l] - xi0) / hz
        j0 = np.clip(np.floor(t).astype(np.int64), 1, J - 3)
        # 4-tap window of B-spline values, quantized to fp8 (at scale S8)
        # with error diffusion along the taps: the taps' coefficients vary
        # smoothly, so pushing each tap's rounding error into the next tap
        # cancels the bulk of the quantization noise in the weighted sum.
        acc = np.zeros(NUM_ENT)
        for k in range(4):
            v = _bspline3(t - (j0 - 1 + k)) * S8
            q = _rtn8(v + acc)
            acc = v + acc - q
            Bmat[off + (j0 - 1 + k), eidx] = q
        # refit: per-batch LSQ of the true Gaussian against the *quantized*
        # basis at a subsample of actual entity literal values (this also
        # absorbs the 1/S8 scale into the coefficients)
        Bs = Bmat[off : off + J][:, sidx]
        G = Bs @ Bs.T + 1e-8 * np.eye(J)
        phi = np.exp(
            -(((a_ctr[:, l][:, None] - num_lit[sidx, l][None, :]) / sig[l]) ** 2)
        )
        C = np.linalg.solve(G, Bs @ phi.T).T            # [B, J]
        lhsT[off : off + J, :] = (C * w[:, l][:, None]).T.astype(np.float16)

    # append the DistMult rows: score_l = (e1*r) @ E^T
    x = E_weight[e1_idx] * R_weight[r_idx]              # [B, D]
    Bmat[K_phi : K_phi + DIM, :] = _rtn8(E_weight.T * S8)
    lhsT[K_phi : K_phi + DIM, :] = (x.T / S8).astype(np.float16)

    Bmat8 = Bmat.astype(np.float32).astype(FP8)         # exact (pre-rounded)

    # pack lhs chunks (zero-pad K -> kc*128): lhs_pack[p, ck*B+b] = lhsT[ck*128+p, b]
    lhs_pad = np.zeros((kc * 128, B), dtype=np.float16)
    lhs_pad[:K] = lhsT
    lhs_pack = np.ascontiguousarray(
        lhs_pad.reshape(kc, 128, B).transpose(1, 0, 2).reshape(128, kc * B)
    )
    return kc, k_last, Bmat8, lhs_pack


def make_in_maps_from(kc, k_last, Bmat8, lhs_pack):
    plan = _xfer_plan(kc)
    n_single = sum(1 for t, _ in plan if t == "s")
    n_pair = sum(1 for t, _ in plan if t == "p")
    in_maps = []
    for core in range(NCORES):
        sl = slice(core * ESH, (core + 1) * ESH)
        Bc = Bmat8[:, sl]
        singles = np.empty((n_single, 128, ESH), dtype=FP8)
        pairs = np.empty((n_pair, 128, 2 * ESH), dtype=FP8)
        m = {"lhs": lhs_pack}
        if n_single:
            m["Bms"] = singles
        si = pi = 0
        for t, ck in plan:
            if t == "s":
                singles[si] = Bc[ck * 128 : (ck + 1) * 128]
                si += 1
            elif t == "p":
                pairs[pi, :, :ESH] = Bc[ck * 128 : (ck + 1) * 128]
                pairs[pi, :, ESH:] = Bc[(ck + 1) * 128 : (ck + 2) * 128]
                pi += 1
            else:
                m["Bml"] = np.ascontiguousarray(Bc[ck * 128 :])
        if n_pair:
            m["Bmp"] = pairs
        in_maps.append(m)
    return in_maps


def make_in_maps(**inputs):
    kc, k_last, Bmat8, lhs_pack = make_host_data(**inputs)
    return make_in_maps_from(kc, k_last, Bmat8, lhs_pack)


_NC_CACHE = {}


def kernel(**inputs) -> np.ndarray:
    kc, k_last, Bmat8, lhs_pack = make_host_data(**inputs)
    if (kc, k_last) not in _NC_CACHE:
        _NC_CACHE[(kc, k_last)] = build_nc(kc, k_last)
    nc = _NC_CACHE[(kc, k_last)]
    in_maps = make_in_maps_from(kc, k_last, Bmat8, lhs_pack)
    res = run_bass_kernel_spmd(nc, in_maps, list(range(NCORES)))
    # per-core out is [2, B, ESH//2] (entity halves); reassemble to [B, ESH]
    out = np.concatenate(
        [
            np.concatenate(
                [np.asarray(res.results[i]["out"][h]) for h in range(2)], axis=1
            )
            for i in range(NCORES)
        ],
        axis=1,
    )
    return out.astype(np.float32)


# revision 15
# speedup vs baseline: 1.0342x; 1.0020x over previous
"""DistMult+KBLN scoring kernel for 8 Trainium2 NeuronCores.

Math (eval mode, per reference):
    e1 = E[e1_idx]; r = R[r_idx]                       [B, D]
    score_l[b,e] = sum_d (e1*r)[b,d] * E[e,d]
    score_n[b,e] = sum_l nf[r_idx][b,l] * exp(-((n_h[b,l]-num_lit[e,l]-c[l])^2/var[l]))
    out = sigmoid(score_l + score_n)                   [B, E]

Key idea: the RBF factor phi(a - m) is a smooth 1-D Gaussian in the
num_lit value m, so expand it per-literal in a cubic B-spline basis on a
knot grid over m (spacing H*sigma_l):

    phi(a[b,l] - m[e,l]) ~= sum_j c_j(a[b,l]) * B3((m[e,l]-xi_j)/h_l)

Then score_n[b,e] = sum_{l,j} (w[b,l]*c_{l,j}) * Bmat[(l,j), e] is ONE
matmul with contraction dim K ~= 1650 (score_l's 200 E-rows are appended
to the same contraction).  The basis matrix is stored in FP8 (E3M4) with
error-diffusion rounding along the 4-tap spline window (the taps' errors
cancel against the smooth spline coefficients), and the per-batch
coefficients are least-squares refit against the *quantized* basis
sampled at actual entity literal values.  The stationary lhs stays FP16
(the PE supports fp16-stationary x fp8-moving matmuls).  The device
kernel is a pure streaming matmul: ~8 MB of fp8 basis per core streamed
from HBM at full DMA bandwidth, accumulated in PSUM (10 sub-accumulators
as halves of 5 banks), sigmoid on the way out, fp16 store.

Sharding: entity axis split row-wise across 8 cores (5000 entities
each); host concatenates. No collectives.
"""
import sys

if "/opt/trn_rl_repo" not in sys.path:
    sys.path.insert(0, "/opt/trn_rl_repo")

import ml_dtypes
import numpy as np

import concourse.bass as bass
import concourse.mybir as mybir
import concourse.tile as _tile
from concourse import tile
from concourse.bass_utils import run_bass_kernel_spmd
from concourse.vector_clock import ScopedClock

B = 64
NUM_ENT = 40000
NUM_REL = 1345
DIM = 200
N_LIT = 100
NCORES = 8
ESH = NUM_ENT // NCORES  # 5000 entities per core

SUBW = 500   # matmul free dim (one PSUM bank holds 512 fp32)
H = 0.61     # B-spline knot spacing in units of sigma_l
GUARD = 4    # extra knots beyond 1/h (cubic overhang)
S8 = 16.0    # fp8 scale: basis values stored as fp8(S8*B3), lhs carries 1/S8
FP8 = ml_dtypes.float8_e3m4

f32 = mybir.dt.float32
f16 = mybir.dt.float16
f8 = mybir.dt.float8e3
AF = mybir.ActivationFunctionType


def _drain_and_barrier_split(self, tick_clock, wait_clock):
    # This walrus build rejects >1 sync-wait per instruction; the tail Drain
    # normally carries one wait per active processor. Collect them on a probe
    # NOP instead (split later by _split_multi_waits) and emit a clean drain.
    nc = self.nc
    probe = nc.sync.nop(nofuse=True, hint="tail_wait_probe")
    wait_clock.add_sem_waits(probe.ins, ScopedClock({None: tick_clock.global_clock}))
    nc.sync.drain()
    nc.all_engine_barrier()
    assert self.sems is not None
    popped = nc._tile_sem_poison_stack.pop()
    assert popped is self._sem_poison
    nc.clear_and_free_semaphores(list(self.sems.allocated().values()))
    nc.all_engine_barrier()


_tile.TileContext._drain_and_barrier = _drain_and_barrier_split


def _split_multi_waits(nc: bass.Bass) -> int:
    """Hoist all-but-one sync wait from every instruction onto standalone
    single-wait EventSemaphore instructions inserted just before it (same
    engine, same block). Needed because this walrus build errors with
    "Too many sync wait commands" on instructions carrying >1 wait."""
    n_split = 0
    for bb in nc.m.functions[0].blocks:
        new_insts = []
        for inst in bb.instructions:
            waits = list(inst.sync_info.on_wait) if inst.sync_info else []
            if len(waits) > 1:
                for sw in waits[:-1]:
                    ev = mybir.InstEventSemaphore(
                        name=nc.get_next_instruction_name(),
                        engine=inst.engine,
                        ins=[],
                        outs=[],
                        sync_info=mybir.SyncInfo(on_wait=[sw], on_update=[]),
                    )
                    nc.register_instruction(ev)
                    new_insts.append(ev)
                    n_split += 1
                inst.sync_info.on_wait = waits[-1:]
            new_insts.append(inst)
        bb.instructions[:] = new_insts
    return n_split


def _xfer_plan(kc: int):
    """DMA transfer plan over chunks: ('p', ck) = pair (ck, ck+1) of full
    128-row chunks, ('s', ck) = one leftover full single, ('l', kc-1) =
    partial last chunk. Pairs give 10KB descriptor lines (the 16 DMA
    engines are latency-bound ~376ns/descriptor below ~10KB)."""
    n_full = kc - 1
    plan = []
    ck = 0
    while ck + 1 < n_full:
        plan.append(("p", ck))
        ck += 2
    if ck < n_full:
        plan.append(("s", ck))
    plan.append(("l", kc - 1))
    return plan


def build_nc(kc: int, k_last: int) -> bass.Bass:
    """kc = number of contraction chunks; last chunk has k_last (<=128) rows.

    All chunk tiles are resident in SBUF (no buffer reuse), and all input
    DMAs are issued upfront so the 16 DMA engines never starve."""
    nc = bass.Bass()
    plan = _xfer_plan(kc)
    n_single = sum(1 for t, _ in plan if t == "s")
    n_pair = sum(1 for t, _ in plan if t == "p")

    if n_single:
        Bms_d = nc.dram_tensor("Bms", [n_single, 128, ESH], f8,
                               kind="ExternalInput")
    if n_pair:
        Bmp_d = nc.dram_tensor("Bmp", [n_pair, 128, 2 * ESH], f8,
                               kind="ExternalInput")
    Bml_d = nc.dram_tensor("Bml", [k_last, ESH], f8, kind="ExternalInput")
    lhs_d = nc.dram_tensor("lhs", [128, kc * B], f16, kind="ExternalInput")
    # out[half, b, x] = sigmoid score for entity half*HALF+x -> 5000B dram
    # lines per (half, b), so the stores use big contiguous descriptors
    out_d = nc.dram_tensor("out", [2, B, ESH // 2], f16, kind="ExternalOutput")

    HALF = ESH // 2   # 2500
    NS = ESH // SUBW  # 10 sub-accumulators; 2 per PSUM bank (split partitions)

    with tile.TileContext(nc) as tc:
        with (
            tc.tile_pool(name="const", bufs=1) as cpool,
            tc.tile_pool(name="ps", bufs=1, space=bass.MemorySpace.PSUM) as pspool,
            tc.tile_pool(name="acc", bufs=1) as accpool,
        ):
            lhs_sb = cpool.tile([128, kc * B], f16, tag="lhs")
            warm = cpool.tile([128, 1], f32, tag="warm")
            out2 = accpool.tile([128, HALF], f16, tag="outsb")

            # lhs through SWDGE (spread over 16 engines; it gates the
            # first matmul so it must land fast), issued before chunk 0
            nc.gpsimd.dma_start(lhs_sb[:], lhs_d[:])
            # load the Sigmoid act table early (it's the only table this
            # kernel uses, so the tail pays no table switch)
            nc.scalar.activation(warm[:], warm[:], AF.Sigmoid)

            # all chunk DMAs upfront on gpsimd = SWDGE (descriptors spread
            # across all 16 DMA engines)
            chunk_src = {}   # ck -> (tile, col_off, rows)
            si = pi = 0
            for t, ck in plan:
                if t == "p":
                    tl = cpool.tile([128, 2 * ESH], f8, tag=f"btp{pi}")
                    nc.gpsimd.dma_start(tl[:], Bmp_d[pi, :, :])
                    chunk_src[ck] = (tl, 0, 128)
                    chunk_src[ck + 1] = (tl, ESH, 128)
                    pi += 1
                elif t == "s":
                    tl = cpool.tile([128, ESH], f8, tag=f"bts{si}")
                    nc.gpsimd.dma_start(tl[:], Bms_d[si, :, :])
                    chunk_src[ck] = (tl, 0, 128)
                    si += 1
                else:
                    tl = cpool.tile([128, ESH], f8, tag="btl")
                    nc.gpsimd.dma_start(tl[0:k_last, :], Bml_d[:])
                    chunk_src[ck] = (tl, 0, k_last)

            ps = [
                pspool.tile([128, SUBW], f32, tag=f"ps{s}", name=f"ps_{s}")
                for s in range(NS // 2)
            ]

            def acc_mm(ck, s, start, stop):
                t, off, rows = chunk_src[ck]
                bank, half = s % 5, s // 5
                nc.tensor.matmul(
                    ps[bank][half * B : (half + 1) * B, :],
                    lhs_sb[0:rows, ck * B : (ck + 1) * B],
                    t[0:rows, off + s * SUBW : off + (s + 1) * SUBW],
                    start=start, stop=stop, tile_position=(0, half * B),
                )

            # chunks 0..kc-3: plain order
            for ck in range(kc - 2):
                for s in range(NS):
                    acc_mm(ck, s, start=(ck == 0), stop=False)
            # final two chunks interleaved bank-major so each PSUM bank's
            # accumulation STOPS early and its sigmoid pipelines under the
            # remaining matmuls (sigmoid chain ends ~0.7us after last matmul)
            for s in (0, 5, 1, 6, 2, 7, 3, 8, 4, 9):
                acc_mm(kc - 2, s, start=False, stop=False)
                acc_mm(kc - 1, s, start=False, stop=True)

            # tail: per PSUM bank, sigmoid straight from PSUM into fp16
            # (rows 0:64 = entity half [0,2500), rows 64:128 = [2500,5000));
            # thanks to the bank-major interleave above the sigmoid chain
            # pipelines under the final matmuls, so the first store (banks
            # 0-2) launches before the last matmul even ends and overlaps
            # the remaining sigmoids + second store
            for bank in range(NS // 2):
                c0 = bank * SUBW
                nc.scalar.activation(
                    out2[:, c0 : c0 + SUBW], ps[bank][:], AF.Sigmoid,
                )
                if bank == 2:
                    nc.gpsimd.dma_start(
                        out_d[:, :, 0 : 3 * SUBW], out2[:, 0 : 3 * SUBW]
                    )
            nc.gpsimd.dma_start(
                out_d[:, :, 3 * SUBW : HALF], out2[:, 3 * SUBW : HALF]
            )

    _split_multi_waits(nc)
    return nc


def _bspline3(t):
    at = np.abs(t)
    r = np.zeros_like(at)
    m1 = at < 1
    r[m1] = (4 - 6 * at[m1] ** 2 + 3 * at[m1] ** 3) / 6
    m2 = (at >= 1) & (at < 2)
    r[m2] = ((2 - at[m2]) ** 3) / 6
    return r


def _rtn8(x):
    """Round f64 -> fp8e3 representable, returned as f64 (same scale)."""
    return np.asarray(x, dtype=np.float32).astype(FP8).astype(np.float64)


def make_host_data(e1_idx, r_idx, E_weight, R_weight, num_lit, c, var, nf_weights):
    """Build the fp8 basis matrix [K, NUM_ENT] and packed fp16 lhs."""
    e1_idx = np.asarray(e1_idx).astype(np.int64)
    r_idx = np.asarray(r_idx).astype(np.int64)
    E_weight = np.asarray(E_weight, dtype=np.float64)
    R_weight = np.asarray(R_weight, dtype=np.float64)
    num_lit = np.asarray(num_lit, dtype=np.float64)
    c = np.asarray(c, dtype=np.float64)
    var = np.asarray(var, dtype=np.float64)
    nf = np.asarray(nf_weights, dtype=np.float64)

    sig = np.sqrt(var)                  # [L]
    a_ctr = num_lit[e1_idx] - c         # [B, L] Gaussian centers, z units
    w = nf[r_idx]                       # [B, L]

    Js = np.array([int(np.ceil(1.0 / (H * s))) + GUARD for s in sig])
    offs = np.concatenate([[0], np.cumsum(Js)[:-1]])
    K_phi = int(Js.sum())
    K = K_phi + DIM
    kc = (K + 127) // 128
    k_last = K - (kc - 1) * 128

    Bmat = np.zeros((K, NUM_ENT), dtype=np.float64)
    lhsT = np.zeros((K, B), dtype=np.float16)
    eidx = np.arange(NUM_ENT)
    sidx = np.arange(0, NUM_ENT, NUM_ENT // 4000)   # refit subsample
    for l in range(N_LIT):
        hz = H * sig[l]
        J = int(Js[l])
        off = int(offs[l])
        xi0 = -2 * hz
        t = (num_lit[:, l] - xi0) / hz
        j0 = np.clip(np.floor(t).astype(np.int64), 1, J - 3)
        # 4-tap window of B-spline values, quantized to fp8 (at scale S8)
        # with error diffusion along the taps: the taps' coefficients vary
        # smoothly, so pushing each tap's rounding error into the next tap
        # cancels the bulk of the quantization noise in the weighted sum.
        acc = np.zeros(NUM_ENT)
        for k in range(4):
            v = _bspline3(t - (j0 - 1 + k)) * S8
            q = _rtn8(v + acc)
            acc = v + acc - q
            Bmat[off + (j0 - 1 + k), eidx] = q
        # refit: per-batch LSQ of the true Gaussian against the *quantized*
        # basis at a subsample of actual entity literal values (this also
        # absorbs the 1/S8 scale into the coefficients)
        Bs = Bmat[off : off + J][:, sidx]
        G = Bs @ Bs.T + 1e-8 * np.eye(J)
        phi = np.exp(
            -(((a_ctr[:, l][:, None] - num_lit[sidx, l][None, :]) / sig[l]) ** 2)
        )
        C = np.linalg.solve(G, Bs @ phi.T).T            # [B, J]
        lhsT[off : off + J, :] = (C * w[:, l][:, None]).T.astype(np.float16)

    # append the DistMult rows: score_l = (e1*r) @ E^T
    x = E_weight[e1_idx] * R_weight[r_idx]              # [B, D]
    Bmat[K_phi : K_phi + DIM, :] = _rtn8(E_weight.T * S8)
    lhsT[K_phi : K_phi + DIM, :] = (x.T / S8).astype(np.float16)

    Bmat8 = Bmat.astype(np.float32).astype(FP8)         # exact (pre-rounded)

    # pack lhs chunks (zero-pad K -> kc*128): lhs_pack[p, ck*B+b] = lhsT[ck*128+p, b]
    lhs_pad = np.zeros((kc * 128, B), dtype=np.float16)
    lhs_pad[:K] = lhsT
    lhs_pack = np.ascontiguousarray(
        lhs_pad.reshape(kc, 128, B).transpose(1, 0, 2).reshape(128, kc * B)
    )
    return kc, k_last, Bmat8, lhs_pack


def make_in_maps_from(kc, k_last, Bmat8, lhs_pack):
    plan = _xfer_plan(kc)
    n_single = sum(1 for t, _ in plan if t == "s")
    n_pair = sum(1 for t, _ in plan if t == "p")
    in_maps = []
    for core in range(NCORES):
        sl = slice(core * ESH, (core + 1) * ESH)
        Bc = Bmat8[:, sl]
        singles = np.empty((n_single, 128, ESH), dtype=FP8)
        pairs = np.empty((n_pair, 128, 2 * ESH), dtype=FP8)
        m = {"lhs": lhs_pack}
        if n_single:
            m["Bms"] = singles
        si = pi = 0
        for t, ck in plan:
            if t == "s":
                singles[si] = Bc[ck * 128 : (ck + 1) * 128]
                si += 1
            elif t == "p":
                pairs[pi, :, :ESH] = Bc[ck * 128 : (ck + 1) * 128]
                pairs[pi, :, ESH:] = Bc[(ck + 1) * 128 : (ck + 2) * 128]
                pi += 1
            else:
                m["Bml"] = np.ascontiguousarray(Bc[ck * 128 :])
        if n_pair:
            m["Bmp"] = pairs
        in_maps.append(m)
    return in_maps


def make_in_maps(**inputs):
    kc, k_last, Bmat8, lhs_pack = make_host_data(**inputs)
    return make_in_maps_from(kc, k_last, Bmat8, lhs_pack)


_NC_CACHE = {}


def kernel(**inputs) -> np.ndarray:
    kc, k_last, Bmat8, lhs_pack = make_host_data(**inputs)
    if (kc, k_last) not in _NC_CACHE:
        _NC_CACHE[(kc, k_last)] = build_nc(kc, k_last)
    nc = _NC_CACHE[(kc, k_last)]
    in_maps = make_in_maps_from(kc, k_last, Bmat8, lhs_pack)
    res = run_bass_kernel_spmd(nc, in_maps, list(range(NCORES)))
    # per-core out is [2, B, ESH//2] (entity halves); reassemble to [B, ESH]
    out = np.concatenate(
        [
            np.concatenate(
                [np.asarray(res.results[i]["out"][h]) for h in range(2)], axis=1
            )
            for i in range(NCORES)
        ],
        axis=1,
    )
    return out.astype(np.float32)


# revision 20
# speedup vs baseline: 1.0803x; 1.0446x over previous
"""DistMult+KBLN scoring kernel for 8 Trainium2 NeuronCores.

Math (eval mode, per reference):
    e1 = E[e1_idx]; r = R[r_idx]                       [B, D]
    score_l[b,e] = sum_d (e1*r)[b,d] * E[e,d]
    score_n[b,e] = sum_l nf[r_idx][b,l] * exp(-((n_h[b,l]-num_lit[e,l]-c[l])^2/var[l]))
    out = sigmoid(score_l + score_n)                   [B, E]

Key idea: the RBF factor phi(a - m) is a smooth 1-D Gaussian in the
num_lit value m, so expand it per-literal in a cubic B-spline basis on a
knot grid over m (spacing H*sigma_l):

    phi(a[b,l] - m[e,l]) ~= sum_j c_j(a[b,l]) * B3((m[e,l]-xi_j)/h_l)

Then score_n[b,e] = sum_{l,j} (w[b,l]*c_{l,j}) * Bmat[(l,j), e] is ONE
matmul with contraction dim K ~= 1650 (score_l's 200 E-rows are appended
to the same contraction).  The basis matrix is stored in FP8 (E3M4) with
error-diffusion rounding along the 4-tap spline window (the taps' errors
cancel against the smooth spline coefficients), and the per-batch
coefficients are least-squares refit against the *quantized* basis
sampled at actual entity literal values.  The stationary lhs stays FP16
(the PE supports fp16-stationary x fp8-moving matmuls).  The device
kernel is a pure streaming matmul: ~8 MB of fp8 basis per core streamed
from HBM at full DMA bandwidth, accumulated in PSUM (10 sub-accumulators
as halves of 5 banks), sigmoid on the way out, fp16 store.

Sharding: entity axis split row-wise across 8 cores (5000 entities
each); host concatenates. No collectives.
"""
import sys

if "/opt/trn_rl_repo" not in sys.path:
    sys.path.insert(0, "/opt/trn_rl_repo")

import ml_dtypes
import numpy as np

import concourse.bass as bass
import concourse.mybir as mybir
import concourse.tile as _tile
from concourse import tile
from concourse.bass_utils import run_bass_kernel_spmd
from concourse.vector_clock import ScopedClock

B = 64
NUM_ENT = 40000
NUM_REL = 1345
DIM = 200
N_LIT = 100
NCORES = 8
ESH = NUM_ENT // NCORES  # 5000 entities per core

SUBW = 500   # matmul free dim (one PSUM bank holds 512 fp32)
H = 0.61     # B-spline knot spacing in units of sigma_l
GUARD = 4    # extra knots beyond 1/h (cubic overhang)
S8 = 16.0    # fp8 scale: basis values stored as fp8(S8*B3), lhs carries 1/S8
FP8 = ml_dtypes.float8_e3m4

f32 = mybir.dt.float32
f16 = mybir.dt.float16
f8 = mybir.dt.float8e3
AF = mybir.ActivationFunctionType


def _drain_and_barrier_split(self, tick_clock, wait_clock):
    # This walrus build rejects >1 sync-wait per instruction; the tail Drain
    # normally carries one wait per active processor. Collect them on a probe
    # NOP instead (split later by _split_multi_waits) and emit a clean drain.
    nc = self.nc
    probe = nc.sync.nop(nofuse=True, hint="tail_wait_probe")
    wait_clock.add_sem_waits(probe.ins, ScopedClock({None: tick_clock.global_clock}))
    nc.sync.drain()
    nc.all_engine_barrier()
    assert self.sems is not None
    popped = nc._tile_sem_poison_stack.pop()
    assert popped is self._sem_poison
    nc.clear_and_free_semaphores(list(self.sems.allocated().values()))
    nc.all_engine_barrier()


_tile.TileContext._drain_and_barrier = _drain_and_barrier_split


def _split_multi_waits(nc: bass.Bass) -> int:
    """Hoist all-but-one sync wait from every instruction onto standalone
    single-wait EventSemaphore instructions inserted just before it (same
    engine, same block). Needed because this walrus build errors with
    "Too many sync wait commands" on instructions carrying >1 wait."""
    n_split = 0
    for bb in nc.m.functions[0].blocks:
        new_insts = []
        for inst in bb.instructions:
            waits = list(inst.sync_info.on_wait) if inst.sync_info else []
            if len(waits) > 1:
                for sw in waits[:-1]:
                    ev = mybir.InstEventSemaphore(
                        name=nc.get_next_instruction_name(),
                        engine=inst.engine,
                        ins=[],
                        outs=[],
                        sync_info=mybir.SyncInfo(on_wait=[sw], on_update=[]),
                    )
                    nc.register_instruction(ev)
                    new_insts.append(ev)
                    n_split += 1
                inst.sync_info.on_wait = waits[-1:]
            new_insts.append(inst)
        bb.instructions[:] = new_insts
    return n_split


def _xfer_plan(kc: int):
    """DMA transfer plan over 128-row chunks (the last one zero-padded to
    128 rows on the host): pair 0 carries the fp16 lhs bytes appended to
    its lines ('0'), the final three chunks ride one triple-wide transfer
    ('t'), the rest go in pairs ('p'). Wide lines matter: the 16 DMA
    engines are latency-bound ~376ns/descriptor below ~10KB."""
    assert kc >= 5 and kc % 2 == 1
    plan = [("0", 0)]
    ck = 2
    while ck + 2 < kc - 1:
        plan.append(("p", ck))
        ck += 2
    plan.append(("t", kc - 3))
    return plan


def build_nc(kc: int) -> bass.Bass:
    """kc = number of 128-row contraction chunks (last zero-padded on host).

    All chunk tiles are resident in SBUF (no buffer reuse), and all input
    DMAs are issued upfront so the 16 DMA engines never starve. The fp16
    lhs bytes ride appended to pair 0's lines (a standalone lhs transfer
    has 1664B lines and burns ~3us of latency-bound descriptor time at
    the queue head); matmuls read them through a bitcast AP."""
    nc = bass.Bass()
    plan = _xfer_plan(kc)
    n_pair = sum(1 for t, _ in plan if t == "p")
    LB = kc * B * 2          # lhs bytes per partition line
    LOFF = 2 * ESH           # lhs byte offset within pair-0 lines

    Bmp0_d = nc.dram_tensor("Bmp0", [128, 2 * ESH + LB], f8,
                            kind="ExternalInput")
    if n_pair:
        Bmp_d = nc.dram_tensor("Bmp", [n_pair, 128, 2 * ESH], f8,
                               kind="ExternalInput")
    Bmt_d = nc.dram_tensor("Bmt", [128, 3 * ESH], f8, kind="ExternalInput")
    # out[half, b, x] = sigmoid score for entity half*HALF+x -> 5000B dram
    # lines per (half, b), so the stores use big contiguous descriptors
    out_d = nc.dram_tensor("out", [2, B, ESH // 2], f16, kind="ExternalOutput")

    HALF = ESH // 2   # 2500
    NS = ESH // SUBW  # 10 sub-accumulators; 2 per PSUM bank (split partitions)

    with tile.TileContext(nc) as tc:
        with (
            tc.tile_pool(name="const", bufs=1) as cpool,
            tc.tile_pool(name="ps", bufs=1, space=bass.MemorySpace.PSUM) as pspool,
            tc.tile_pool(name="acc", bufs=1) as accpool,
        ):
            warm = cpool.tile([128, 1], f32, tag="warm")
            out2 = accpool.tile([128, HALF], f16, tag="outsb")

            # load the Sigmoid act table early (it's the only table this
            # kernel uses, so the tail pays no table switch)
            nc.scalar.activation(warm[:], warm[:], AF.Sigmoid)

            # all chunk DMAs upfront on gpsimd = SWDGE (descriptors spread
            # across all 16 DMA engines)
            chunk_src = {}   # ck -> (tile, col_off)
            pi = 0
            p0 = None
            for t, ck in plan:
                if t == "0":
                    p0 = cpool.tile([128, 2 * ESH + LB], f8, tag="btp0")
                    nc.gpsimd.dma_start(p0[:], Bmp0_d[:])
                    chunk_src[ck] = (p0, 0)
                    chunk_src[ck + 1] = (p0, ESH)
                elif t == "p":
                    tl = cpool.tile([128, 2 * ESH], f8, tag=f"btp{pi + 1}")
                    nc.gpsimd.dma_start(tl[:], Bmp_d[pi, :, :])
                    chunk_src[ck] = (tl, 0)
                    chunk_src[ck + 1] = (tl, ESH)
                    pi += 1
                else:
                    tl = cpool.tile([128, 3 * ESH], f8, tag="btt")
                    nc.gpsimd.dma_start(tl[:], Bmt_d[:])
                    chunk_src[ck] = (tl, 0)
                    chunk_src[ck + 1] = (tl, ESH)
                    chunk_src[ck + 2] = (tl, 2 * ESH)

            ps = [
                pspool.tile([128, SUBW], f32, tag=f"ps{s}", name=f"ps_{s}")
                for s in range(NS // 2)
            ]

            def acc_mm(ck, s, start, stop):
                t, off = chunk_src[ck]
                bank, half = s % 5, s // 5
                nc.tensor.matmul(
                    ps[bank][half * B : (half + 1) * B, :],
                    p0[:, LOFF + ck * 2 * B : LOFF + (ck + 1) * 2 * B].bitcast(f16),
                    t[:, off + s * SUBW : off + (s + 1) * SUBW],
                    start=start, stop=stop, tile_position=(0, half * B),
                )

            # chunks 0..kc-3: plain order
            for ck in range(kc - 2):
                for s in range(NS):
                    acc_mm(ck, s, start=(ck == 0), stop=False)
            # final two chunks interleaved bank-major so each PSUM bank's
            # accumulation STOPS early and its sigmoid pipelines under the
            # remaining matmuls (sigmoid chain ends ~0.7us after last matmul)
            for s in (0, 5, 1, 6, 2, 7, 3, 8, 4, 9):
                acc_mm(kc - 2, s, start=False, stop=False)
                acc_mm(kc - 1, s, start=False, stop=True)

            # tail: per PSUM bank, sigmoid straight from PSUM into fp16
            # (rows 0:64 = entity half [0,2500), rows 64:128 = [2500,5000));
            # thanks to the bank-major interleave above the sigmoid chain
            # pipelines under the final matmuls, so the first store (banks
            # 0-2) launches before the last matmul even ends and overlaps
            # the remaining sigmoids + second store
            for bank in range(NS // 2):
                c0 = bank * SUBW
                nc.scalar.activation(
                    out2[:, c0 : c0 + SUBW], ps[bank][:], AF.Sigmoid,
                )
                if bank == 2:
                    nc.gpsimd.dma_start(
                        out_d[:, :, 0 : 3 * SUBW], out2[:, 0 : 3 * SUBW]
                    )
            nc.gpsimd.dma_start(
                out_d[:, :, 3 * SUBW : HALF], out2[:, 3 * SUBW : HALF]
            )

    _split_multi_waits(nc)
    return nc


def _bspline3(t):
    at = np.abs(t)
    r = np.zeros_like(at)
    m1 = at < 1
    r[m1] = (4 - 6 * at[m1] ** 2 + 3 * at[m1] ** 3) / 6
    m2 = (at >= 1) & (at < 2)
    r[m2] = ((2 - at[m2]) ** 3) / 6
    return r


def _rtn8(x):
    """Round f64 -> fp8e3 representable, returned as f64 (same scale)."""
    return np.asarray(x, dtype=np.float32).astype(FP8).astype(np.float64)


def make_host_data(e1_idx, r_idx, E_weight, R_weight, num_lit, c, var, nf_weights):
    """Build the fp8 basis matrix [K, NUM_ENT] and packed fp16 lhs."""
    e1_idx = np.asarray(e1_idx).astype(np.int64)
    r_idx = np.asarray(r_idx).astype(np.int64)
    E_weight = np.asarray(E_weight, dtype=np.float64)
    R_weight = np.asarray(R_weight, dtype=np.float64)
    num_lit = np.asarray(num_lit, dtype=np.float64)
    c = np.asarray(c, dtype=np.float64)
    var = np.asarray(var, dtype=np.float64)
    nf = np.asarray(nf_weights, dtype=np.float64)

    sig = np.sqrt(var)                  # [L]
    a_ctr = num_lit[e1_idx] - c         # [B, L] Gaussian centers, z units
    w = nf[r_idx]                       # [B, L]

    Js = np.array([int(np.ceil(1.0 / (H * s))) + GUARD for s in sig])
    offs = np.concatenate([[0], np.cumsum(Js)[:-1]])
    K_phi = int(Js.sum())
    K = K_phi + DIM
    kc = (K + 127) // 128

    Bmat = np.zeros((K, NUM_ENT), dtype=np.float64)
    lhsT = np.zeros((K, B), dtype=np.float16)
    eidx = np.arange(NUM_ENT)
    sidx = np.arange(0, NUM_ENT, NUM_ENT // 4000)   # refit subsample
    for l in range(N_LIT):
        hz = H * sig[l]
        J = int(Js[l])
        off = int(offs[l])
        xi0 = -2 * hz
        t = (num_lit[:, l] - xi0) / hz
        j0 = np.clip(np.floor(t).astype(np.int64), 1, J - 3)
        # 4-tap window of B-spline values, quantized to fp8 (at scale S8)
        # with error diffusion along the taps: the taps' coefficients vary
        # smoothly, so pushing each tap's rounding error into the next tap
        # cancels the bulk of the quantization noise in the weighted sum.
        acc = np.zeros(NUM_ENT)
        for k in range(4):
            v = _bspline3(t - (j0 - 1 + k)) * S8
            q = _rtn8(v + acc)
            acc = v + acc - q
            Bmat[off + (j0 - 1 + k), eidx] = q
        # refit: per-batch LSQ of the true Gaussian against the *quantized*
        # basis at a subsample of actual entity literal values (this also
        # absorbs the 1/S8 scale into the coefficients)
        Bs = Bmat[off : off + J][:, sidx]
        G = Bs @ Bs.T + 1e-8 * np.eye(J)
        phi = np.exp(
            -(((a_ctr[:, l][:, None] - num_lit[sidx, l][None, :]) / sig[l]) ** 2)
        )
        C = np.linalg.solve(G, Bs @ phi.T).T            # [B, J]
        lhsT[off : off + J, :] = (C * w[:, l][:, None]).T.astype(np.float16)

    # append the DistMult rows: score_l = (e1*r) @ E^T
    x = E_weight[e1_idx] * R_weight[r_idx]              # [B, D]
    Bmat[K_phi : K_phi + DIM, :] = _rtn8(E_weight.T * S8)
    lhsT[K_phi : K_phi + DIM, :] = (x.T / S8).astype(np.float16)

    Bmat8 = Bmat.astype(np.float32).astype(FP8)         # exact (pre-rounded)

    # pack lhs chunks (zero-pad K -> kc*128): lhs_pack[p, ck*B+b] = lhsT[ck*128+p, b]
    lhs_pad = np.zeros((kc * 128, B), dtype=np.float16)
    lhs_pad[:K] = lhsT
    lhs_pack = np.ascontiguousarray(
        lhs_pad.reshape(kc, 128, B).transpose(1, 0, 2).reshape(128, kc * B)
    )
    return kc, Bmat8, lhs_pack


def make_in_maps_from(kc, Bmat8, lhs_pack):
    plan = _xfer_plan(kc)
    n_pair = sum(1 for t, _ in plan if t == "p")
    K = Bmat8.shape[0]
    lhs_bytes = lhs_pack.view(FP8)           # [128, kc*B*2] raw fp16 bytes
    in_maps = []
    for core in range(NCORES):
        sl = slice(core * ESH, (core + 1) * ESH)
        Bpad = np.zeros((kc * 128, ESH), dtype=FP8)
        Bpad[:K] = Bmat8[:, sl]
        ch = lambda ck: Bpad[ck * 128 : (ck + 1) * 128]
        pairs = np.empty((n_pair, 128, 2 * ESH), dtype=FP8)
        m = {}
        pi = 0
        for t, ck in plan:
            if t == "0":
                m["Bmp0"] = np.ascontiguousarray(
                    np.concatenate([ch(ck), ch(ck + 1), lhs_bytes], axis=1)
                )
            elif t == "p":
                pairs[pi, :, :ESH] = ch(ck)
                pairs[pi, :, ESH:] = ch(ck + 1)
                pi += 1
            else:
                m["Bmt"] = np.ascontiguousarray(
                    np.concatenate([ch(ck), ch(ck + 1), ch(ck + 2)], axis=1)
                )
        if n_pair:
            m["Bmp"] = pairs
        in_maps.append(m)
    return in_maps


def make_in_maps(**inputs):
    kc, Bmat8, lhs_pack = make_host_data(**inputs)
    return make_in_maps_from(kc, Bmat8, lhs_pack)


_NC_CACHE = {}


def kernel(**inputs) -> np.ndarray:
    kc, Bmat8, lhs_pack = make_host_data(**inputs)
    if kc not in _NC_CACHE:
        _NC_CACHE[kc] = build_nc(kc)
    nc = _NC_CACHE[kc]
    in_maps = make_in_maps_from(kc, Bmat8, lhs_pack)
    res = run_bass_kernel_spmd(nc, in_maps, list(range(NCORES)))
    # per-core out is [2, B, ESH//2] (entity halves); reassemble to [B, ESH]
    out = np.concatenate(
        [
            np.concatenate(
                [np.asarray(res.results[i]["out"][h]) for h in range(2)], axis=1
            )
            for i in range(NCORES)
        ],
        axis=1,
    )
    return out.astype(np.float32)


# revision 25
# speedup vs baseline: 1.1049x; 1.0227x over previous
"""DistMult+KBLN scoring kernel for 8 Trainium2 NeuronCores.

Math (eval mode, per reference):
    e1 = E[e1_idx]; r = R[r_idx]                       [B, D]
    score_l[b,e] = sum_d (e1*r)[b,d] * E[e,d]
    score_n[b,e] = sum_l nf[r_idx][b,l] * exp(-((n_h[b,l]-num_lit[e,l]-c[l])^2/var[l]))
    out = sigmoid(score_l + score_n)                   [B, E]

Key idea: the RBF factor phi(a - m) is a smooth 1-D Gaussian in the
num_lit value m, so expand it per-literal in a cubic B-spline basis on a
knot grid over m (spacing H*sigma_l):

    phi(a[b,l] - m[e,l]) ~= sum_j c_j(a[b,l]) * B3((m[e,l]-xi_j)/h_l)

Then score_n[b,e] = sum_{l,j} (w[b,l]*c_{l,j}) * Bmat[(l,j), e] is ONE
matmul with contraction dim K ~= 1650 (score_l's 200 E-rows are appended
to the same contraction).  The basis matrix is stored in FP8 (E3M4) with
error-diffusion rounding along the 4-tap spline window (the taps' errors
cancel against the smooth spline coefficients), and the per-batch
coefficients are least-squares refit against the *quantized* basis
sampled at actual entity literal values.  The stationary lhs stays FP16
(the PE supports fp16-stationary x fp8-moving matmuls).  The device
kernel is a pure streaming matmul: ~8 MB of fp8 basis per core streamed
from HBM at full DMA bandwidth, accumulated in PSUM (10 sub-accumulators
as halves of 5 banks), sigmoid on the way out, fp16 store.

Sharding: entity axis split row-wise across 8 cores (5000 entities
each); host concatenates. No collectives.
"""
import sys

if "/opt/trn_rl_repo" not in sys.path:
    sys.path.insert(0, "/opt/trn_rl_repo")

import ml_dtypes
import numpy as np

import concourse.bass as bass
import concourse.mybir as mybir
import concourse.tile as _tile
from concourse import tile
from concourse.bass_utils import run_bass_kernel_spmd
from concourse.vector_clock import ScopedClock

B = 64
NUM_ENT = 40000
NUM_REL = 1345
DIM = 200
N_LIT = 100
NCORES = 8
ESH = NUM_ENT // NCORES  # 5000 entities per core

SUBW = 500   # matmul free dim (one PSUM bank holds 512 fp32)
H = 0.69     # B-spline knot spacing in units of sigma_l
GUARD = 4    # extra knots beyond 1/h (cubic overhang)
S8 = 16.0    # fp8 scale: basis values stored as fp8(S8*B3), lhs carries 1/S8
FP8 = ml_dtypes.float8_e3m4

f32 = mybir.dt.float32
f16 = mybir.dt.float16
f8 = mybir.dt.float8e3
AF = mybir.ActivationFunctionType


def _drain_and_barrier_split(self, tick_clock, wait_clock):
    # This walrus build rejects >1 sync-wait per instruction; the tail Drain
    # normally carries one wait per active processor. Collect them on a probe
    # NOP instead (split later by _split_multi_waits) and emit a clean drain.
    nc = self.nc
    probe = nc.sync.nop(nofuse=True, hint="tail_wait_probe")
    wait_clock.add_sem_waits(probe.ins, ScopedClock({None: tick_clock.global_clock}))
    nc.sync.drain()
    nc.all_engine_barrier()
    assert self.sems is not None
    popped = nc._tile_sem_poison_stack.pop()
    assert popped is self._sem_poison
    nc.clear_and_free_semaphores(list(self.sems.allocated().values()))
    nc.all_engine_barrier()


_tile.TileContext._drain_and_barrier = _drain_and_barrier_split


def _split_multi_waits(nc: bass.Bass) -> int:
    """Hoist all-but-one sync wait from every instruction onto standalone
    single-wait EventSemaphore instructions inserted just before it (same
    engine, same block). Needed because this walrus build errors with
    "Too many sync wait commands" on instructions carrying >1 wait."""
    n_split = 0
    for bb in nc.m.functions[0].blocks:
        new_insts = []
        for inst in bb.instructions:
            waits = list(inst.sync_info.on_wait) if inst.sync_info else []
            if len(waits) > 1:
                for sw in waits[:-1]:
                    ev = mybir.InstEventSemaphore(
                        name=nc.get_next_instruction_name(),
                        engine=inst.engine,
                        ins=[],
                        outs=[],
                        sync_info=mybir.SyncInfo(on_wait=[sw], on_update=[]),
                    )
                    nc.register_instruction(ev)
                    new_insts.append(ev)
                    n_split += 1
                inst.sync_info.on_wait = waits[-1:]
            new_insts.append(inst)
        bb.instructions[:] = new_insts
    return n_split


def _xfer_plan(kc: int):
    """DMA transfer plan over 128-row chunks (the last one zero-padded to
    128 rows on the host): pair 0 carries the fp16 lhs bytes appended to
    its lines ('0'), the final three chunks ride one triple-wide transfer
    ('t'), the rest go in pairs ('p'). Wide lines matter: the 16 DMA
    engines are latency-bound ~376ns/descriptor below ~10KB."""
    assert kc >= 5
    plan = [("0", 0)]
    ck = 2
    while kc - ck >= 5:
        plan.append(("p", ck))
        ck += 2
    if kc - ck == 3:
        plan.append(("t", ck))
    else:
        plan.append(("p", ck))
        plan.append(("p", ck + 2))
    return plan


def build_nc(kc: int) -> bass.Bass:
    """kc = number of 128-row contraction chunks (last zero-padded on host).

    All chunk tiles are resident in SBUF (no buffer reuse), and all input
    DMAs are issued upfront so the 16 DMA engines never starve. The fp16
    lhs bytes ride appended to pair 0's lines (a standalone lhs transfer
    has 1664B lines and burns ~3us of latency-bound descriptor time at
    the queue head); matmuls read them through a bitcast AP."""
    nc = bass.Bass()
    plan = _xfer_plan(kc)
    n_pair = sum(1 for t, _ in plan if t == "p")
    LB = kc * B * 2          # lhs bytes per partition line
    LOFF = 2 * ESH           # lhs byte offset within pair-0 lines

    n_trip = sum(1 for t, _ in plan if t == "t")
    Bmp0_d = nc.dram_tensor("Bmp0", [128, 2 * ESH + LB], f8,
                            kind="ExternalInput")
    if n_pair:
        Bmp_d = nc.dram_tensor("Bmp", [n_pair, 128, 2 * ESH], f8,
                               kind="ExternalInput")
    if n_trip:
        Bmt_d = nc.dram_tensor("Bmt", [128, 3 * ESH], f8, kind="ExternalInput")
    # out[half, b, x] = sigmoid score for entity half*HALF+x -> 5000B dram
    # lines per (half, b), so the stores use big contiguous descriptors
    out_d = nc.dram_tensor("out", [2, B, ESH // 2], f16, kind="ExternalOutput")

    HALF = ESH // 2   # 2500
    NS = ESH // SUBW  # 10 sub-accumulators; 2 per PSUM bank (split partitions)

    with tile.TileContext(nc) as tc:
        with (
            tc.tile_pool(name="const", bufs=1) as cpool,
            tc.tile_pool(name="ps", bufs=1, space=bass.MemorySpace.PSUM) as pspool,
            tc.tile_pool(name="acc", bufs=1) as accpool,
        ):
            warm = cpool.tile([128, 1], f32, tag="warm")
            out2 = accpool.tile([128, HALF], f16, tag="outsb")

            # load the Sigmoid act table early (it's the only table this
            # kernel uses, so the tail pays no table switch)
            nc.scalar.activation(warm[:], warm[:], AF.Sigmoid)

            # all chunk DMAs upfront on gpsimd = SWDGE (descriptors spread
            # across all 16 DMA engines)
            chunk_src = {}   # ck -> (tile, col_off)
            pi = 0
            p0 = None
            for t, ck in plan:
                if t == "0":
                    p0 = cpool.tile([128, 2 * ESH + LB], f8, tag="btp0")
                    nc.gpsimd.dma_start(p0[:], Bmp0_d[:])
                    chunk_src[ck] = (p0, 0)
                    chunk_src[ck + 1] = (p0, ESH)
                elif t == "p":
                    tl = cpool.tile([128, 2 * ESH], f8, tag=f"btp{pi + 1}")
                    nc.gpsimd.dma_start(tl[:], Bmp_d[pi, :, :])
                    chunk_src[ck] = (tl, 0)
                    chunk_src[ck + 1] = (tl, ESH)
                    pi += 1
                else:
                    tl = cpool.tile([128, 3 * ESH], f8, tag="btt")
                    nc.gpsimd.dma_start(tl[:], Bmt_d[:])
                    chunk_src[ck] = (tl, 0)
                    chunk_src[ck + 1] = (tl, ESH)
                    chunk_src[ck + 2] = (tl, 2 * ESH)

            ps = [
                pspool.tile([128, SUBW], f32, tag=f"ps{s}", name=f"ps_{s}")
                for s in range(NS // 2)
            ]

            def acc_mm(ck, s, start, stop):
                t, off = chunk_src[ck]
                bank, half = s % 5, s // 5
                nc.tensor.matmul(
                    ps[bank][half * B : (half + 1) * B, :],
                    p0[:, LOFF + ck * 2 * B : LOFF + (ck + 1) * 2 * B].bitcast(f16),
                    t[:, off + s * SUBW : off + (s + 1) * SUBW],
                    start=start, stop=stop, tile_position=(0, half * B),
                )

            # chunks 0..kc-3: plain order
            for ck in range(kc - 2):
                for s in range(NS):
                    acc_mm(ck, s, start=(ck == 0), stop=False)
            # final two chunks interleaved bank-major so each PSUM bank's
            # accumulation STOPS early and its sigmoid pipelines under the
            # remaining matmuls (sigmoid chain ends ~0.7us after last matmul)
            for s in (0, 5, 1, 6, 2, 7, 3, 8, 4, 9):
                acc_mm(kc - 2, s, start=False, stop=False)
                acc_mm(kc - 1, s, start=False, stop=True)

            # tail: per PSUM bank, sigmoid straight from PSUM into fp16
            # (rows 0:64 = entity half [0,2500), rows 64:128 = [2500,5000));
            # thanks to the bank-major interleave above the sigmoid chain
            # pipelines under the final matmuls, so the first store (banks
            # 0-2) launches before the last matmul even ends and overlaps
            # the remaining sigmoids + second store
            for bank in range(NS // 2):
                c0 = bank * SUBW
                nc.scalar.activation(
                    out2[:, c0 : c0 + SUBW], ps[bank][:], AF.Sigmoid,
                )
                if bank == 2:
                    nc.gpsimd.dma_start(
                        out_d[:, :, 0 : 3 * SUBW], out2[:, 0 : 3 * SUBW]
                    )
            nc.gpsimd.dma_start(
                out_d[:, :, 3 * SUBW : HALF], out2[:, 3 * SUBW : HALF]
            )

    _split_multi_waits(nc)
    return nc


def _bspline3(t):
    at = np.abs(t)
    r = np.zeros_like(at)
    m1 = at < 1
    r[m1] = (4 - 6 * at[m1] ** 2 + 3 * at[m1] ** 3) / 6
    m2 = (at >= 1) & (at < 2)
    r[m2] = ((2 - at[m2]) ** 3) / 6
    return r


def _rtn8(x):
    """Round f64 -> fp8e3 representable, returned as f64 (same scale)."""
    return np.asarray(x, dtype=np.float32).astype(FP8).astype(np.float64)


def _quant_ed3(vals, M4):
    """vals [4, E]: S8-scaled spline window values. M4 [E, 4, 4]: covariance
    of the w-weighted spline coefficients over each window. Picks, per
    entity, the floor/ceil combo of the 4 taps minimizing eps^T M4 eps
    (the expected squared error of the weighted sum over the batch)."""
    near = np.empty_like(vals)
    alt = np.empty_like(vals)
    for j in range(4):
        v = vals[j]
        q = _rtn8(v)
        step = np.maximum(np.abs(q) * 2.0 ** -4, 2.0 ** -6)
        alt[j] = _rtn8(np.where(q > v, q - step, q + step))
        near[j] = q
    best_cost = np.full(vals.shape[1], np.inf)
    best = np.empty_like(vals)
    for m in range(16):
        bits = np.array([(m >> j) & 1 for j in range(4)])[:, None]
        pick = np.where(bits == 1, alt, near)
        eps = pick - vals
        cost = np.einsum('ae,eab,be->e', eps, M4, eps)
        sel = cost < best_cost
        best_cost = np.where(sel, cost, best_cost)
        best = np.where(sel[None, :], pick, best)
    return best


def make_host_data(e1_idx, r_idx, E_weight, R_weight, num_lit, c, var, nf_weights):
    """Build the fp8 basis matrix [K, NUM_ENT] and packed fp16 lhs."""
    e1_idx = np.asarray(e1_idx).astype(np.int64)
    r_idx = np.asarray(r_idx).astype(np.int64)
    E_weight = np.asarray(E_weight, dtype=np.float64)
    R_weight = np.asarray(R_weight, dtype=np.float64)
    num_lit = np.asarray(num_lit, dtype=np.float64)
    c = np.asarray(c, dtype=np.float64)
    var = np.asarray(var, dtype=np.float64)
    nf = np.asarray(nf_weights, dtype=np.float64)

    sig = np.sqrt(var)                  # [L]
    a_ctr = num_lit[e1_idx] - c         # [B, L] Gaussian centers, z units
    w = nf[r_idx]                       # [B, L]

    Js = np.array([int(np.ceil(1.0 / (H * s))) + GUARD for s in sig])
    offs = np.concatenate([[0], np.cumsum(Js)[:-1]])
    K_phi = int(Js.sum())
    K = K_phi + DIM
    kc = (K + 127) // 128

    Bmat = np.zeros((K, NUM_ENT), dtype=np.float64)
    lhsT = np.zeros((K, B), dtype=np.float16)
    eidx = np.arange(NUM_ENT)
    sidx = np.arange(0, NUM_ENT, NUM_ENT // 4000)   # refit subsample
    for l in range(N_LIT):
        hz = H * sig[l]
        J = int(Js[l])
        off = int(offs[l])
        xi0 = -2 * hz
        t = (num_lit[:, l] - xi0) / hz
        j0 = np.clip(np.floor(t).astype(np.int64), 1, J - 3)
        # 4-tap window of B-spline values, quantized to fp8 (at scale S8)
        # with covariance-weighted rounding: per entity, pick the floor/
        # ceil combo of the 4 taps that minimizes the expected squared
        # error of the coefficient-weighted sum over the batch.
        zfit = np.linspace(-2 * hz, 1 + 2 * hz, 4 * J)
        xi = xi0 + hz * np.arange(J)
        Bz = _bspline3((zfit[:, None] - xi[None, :]) / hz)
        G0 = Bz.T @ Bz + 1e-9 * np.eye(J)
        S0 = np.exp(-(((a_ctr[:, l][:, None] - zfit[None, :]) / sig[l]) ** 2))
        C0 = np.linalg.solve(G0, Bz.T @ S0.T).T * w[:, l][:, None]
        M = C0.T @ C0 / B + 1e-12 * np.eye(J)
        jw = j0[None, :] - 1 + np.arange(4)[:, None]      # [4, E]
        M4 = np.moveaxis(M[jw[:, None, :], jw[None, :, :]], 2, 0)
        win = np.stack([_bspline3(t - (j0 - 1 + k)) for k in range(4)]) * S8
        winq = _quant_ed3(win, M4)
        for k in range(4):
            Bmat[off + (j0 - 1 + k), eidx] = winq[k]
        # refit: per-batch LSQ of the true Gaussian against the *quantized*
        # basis at a subsample of actual entity literal values (this also
        # absorbs the 1/S8 scale into the coefficients)
        Bs = Bmat[off : off + J][:, sidx]
        G = Bs @ Bs.T + 1e-8 * np.eye(J)
        phi = np.exp(
            -(((a_ctr[:, l][:, None] - num_lit[sidx, l][None, :]) / sig[l]) ** 2)
        )
        C = np.linalg.solve(G, Bs @ phi.T).T            # [B, J]
        lhsT[off : off + J, :] = (C * w[:, l][:, None]).T.astype(np.float16)

    # append the DistMult rows: score_l = (e1*r) @ E^T
    x = E_weight[e1_idx] * R_weight[r_idx]              # [B, D]
    Bmat[K_phi : K_phi + DIM, :] = _rtn8(E_weight.T * S8)
    lhsT[K_phi : K_phi + DIM, :] = (x.T / S8).astype(np.float16)

    Bmat8 = Bmat.astype(np.float32).astype(FP8)         # exact (pre-rounded)

    # pack lhs chunks (zero-pad K -> kc*128): lhs_pack[p, ck*B+b] = lhsT[ck*128+p, b]
    lhs_pad = np.zeros((kc * 128, B), dtype=np.float16)
    lhs_pad[:K] = lhsT
    lhs_pack = np.ascontiguousarray(
        lhs_pad.reshape(kc, 128, B).transpose(1, 0, 2).reshape(128, kc * B)
    )
    return kc, Bmat8, lhs_pack


def make_in_maps_from(kc, Bmat8, lhs_pack):
    plan = _xfer_plan(kc)
    n_pair = sum(1 for t, _ in plan if t == "p")
    K = Bmat8.shape[0]
    lhs_bytes = lhs_pack.view(FP8)           # [128, kc*B*2] raw fp16 bytes
    in_maps = []
    for core in range(NCORES):
        sl = slice(core * ESH, (core + 1) * ESH)
        Bpad = np.zeros((kc * 128, ESH), dtype=FP8)
        Bpad[:K] = Bmat8[:, sl]
        ch = lambda ck: Bpad[ck * 128 : (ck + 1) * 128]
        pairs = np.empty((n_pair, 128, 2 * ESH), dtype=FP8)
        m = {}
        pi = 0
        for t, ck in plan:
            if t == "0":
                m["Bmp0"] = np.ascontiguousarray(
                    np.concatenate([ch(ck), ch(ck + 1), lhs_bytes], axis=1)
                )
            elif t == "p":
                pairs[pi, :, :ESH] = ch(ck)
                pairs[pi, :, ESH:] = ch(ck + 1)
                pi += 1
            else:
                m["Bmt"] = np.ascontiguousarray(
                    np.concatenate([ch(ck), ch(ck + 1), ch(ck + 2)], axis=1)
                )
        if n_pair:
            m["Bmp"] = pairs
        in_maps.append(m)
    return in_maps


def make_in_maps(**inputs):
    kc, Bmat8, lhs_pack = make_host_data(**inputs)
    return make_in_maps_from(kc, Bmat8, lhs_pack)


_NC_CACHE = {}


def kernel(**inputs) -> np.ndarray:
    kc, Bmat8, lhs_pack = make_host_data(**inputs)
    if kc not in _NC_CACHE:
        _NC_CACHE[kc] = build_nc(kc)
    nc = _NC_CACHE[kc]
    in_maps = make_in_maps_from(kc, Bmat8, lhs_pack)
    res = run_bass_kernel_spmd(nc, in_maps, list(range(NCORES)))
    # per-core out is [2, B, ESH//2] (entity halves); reassemble to [B, ESH]
    out = np.concatenate(
        [
            np.concatenate(
                [np.asarray(res.results[i]["out"][h]) for h in range(2)], axis=1
            )
            for i in range(NCORES)
        ],
        axis=1,
    )
    return out.astype(np.float32)


# revision 27
# speedup vs baseline: 1.1251x; 1.0183x over previous
"""DistMult+KBLN scoring kernel for 8 Trainium2 NeuronCores.

Math (eval mode, per reference):
    e1 = E[e1_idx]; r = R[r_idx]                       [B, D]
    score_l[b,e] = sum_d (e1*r)[b,d] * E[e,d]
    score_n[b,e] = sum_l nf[r_idx][b,l] * exp(-((n_h[b,l]-num_lit[e,l]-c[l])^2/var[l]))
    out = sigmoid(score_l + score_n)                   [B, E]

Key idea: the RBF factor phi(a - m) is a smooth 1-D Gaussian in the
num_lit value m, so expand it per-literal in a cubic B-spline basis on a
knot grid over m (spacing H*sigma_l):

    phi(a[b,l] - m[e,l]) ~= sum_j c_j(a[b,l]) * B3((m[e,l]-xi_j)/h_l)

Then score_n[b,e] = sum_{l,j} (w[b,l]*c_{l,j}) * Bmat[(l,j), e] is ONE
matmul with contraction dim K ~= 1650 (score_l's 200 E-rows are appended
to the same contraction).  The basis matrix is stored in FP8 (E3M4) with
error-diffusion rounding along the 4-tap spline window (the taps' errors
cancel against the smooth spline coefficients), and the per-batch
coefficients are least-squares refit against the *quantized* basis
sampled at actual entity literal values.  The stationary lhs stays FP16
(the PE supports fp16-stationary x fp8-moving matmuls).  The device
kernel is a pure streaming matmul: ~8 MB of fp8 basis per core streamed
from HBM at full DMA bandwidth, accumulated in PSUM (10 sub-accumulators
as halves of 5 banks), sigmoid on the way out, fp16 store.

Sharding: entity axis split row-wise across 8 cores (5000 entities
each); host concatenates. No collectives.
"""
import sys

if "/opt/trn_rl_repo" not in sys.path:
    sys.path.insert(0, "/opt/trn_rl_repo")

import ml_dtypes
import numpy as np

import concourse.bass as bass
import concourse.mybir as mybir
import concourse.tile as _tile
from concourse import tile
from concourse.bass_utils import run_bass_kernel_spmd
from concourse.vector_clock import ScopedClock

B = 64
NUM_ENT = 40000
NUM_REL = 1345
DIM = 200
N_LIT = 100
NCORES = 8
ESH = NUM_ENT // NCORES  # 5000 entities per core

SUBW = 500   # matmul free dim (one PSUM bank holds 512 fp32)
H = 0.69     # B-spline knot spacing in units of sigma_l
GUARD = 4    # extra knots beyond 1/h (cubic overhang)
S8 = 16.0    # fp8 scale: basis values stored as fp8(S8*B3), lhs carries 1/S8
FP8 = ml_dtypes.float8_e3m4

f32 = mybir.dt.float32
f16 = mybir.dt.float16
f8 = mybir.dt.float8e3
AF = mybir.ActivationFunctionType


def _drain_and_barrier_split(self, tick_clock, wait_clock):
    # This walrus build rejects >1 sync-wait per instruction; the tail Drain
    # normally carries one wait per active processor. Collect them on a probe
    # NOP instead (split later by _split_multi_waits) and emit a clean drain.
    nc = self.nc
    probe = nc.sync.nop(nofuse=True, hint="tail_wait_probe")
    wait_clock.add_sem_waits(probe.ins, ScopedClock({None: tick_clock.global_clock}))
    nc.sync.drain()
    nc.all_engine_barrier()
    assert self.sems is not None
    popped = nc._tile_sem_poison_stack.pop()
    assert popped is self._sem_poison
    nc.clear_and_free_semaphores(list(self.sems.allocated().values()))
    nc.all_engine_barrier()


_tile.TileContext._drain_and_barrier = _drain_and_barrier_split


def _split_multi_waits(nc: bass.Bass) -> int:
    """Hoist all-but-one sync wait from every instruction onto standalone
    single-wait EventSemaphore instructions inserted just before it (same
    engine, same block). Needed because this walrus build errors with
    "Too many sync wait commands" on instructions carrying >1 wait."""
    n_split = 0
    for bb in nc.m.functions[0].blocks:
        new_insts = []
        for inst in bb.instructions:
            waits = list(inst.sync_info.on_wait) if inst.sync_info else []
            if len(waits) > 1:
                for sw in waits[:-1]:
                    ev = mybir.InstEventSemaphore(
                        name=nc.get_next_instruction_name(),
                        engine=inst.engine,
                        ins=[],
                        outs=[],
                        sync_info=mybir.SyncInfo(on_wait=[sw], on_update=[]),
                    )
                    nc.register_instruction(ev)
                    new_insts.append(ev)
                    n_split += 1
                inst.sync_info.on_wait = waits[-1:]
            new_insts.append(inst)
        bb.instructions[:] = new_insts
    return n_split


def _xfer_plan(kc: int):
    """DMA transfer plan over 128-row chunks (the last one zero-padded to
    128 rows on the host): pair 0 carries the fp16 lhs bytes appended to
    its lines ('0'), the final three chunks ride one triple-wide transfer
    ('t'), the rest go in pairs ('p'). Wide lines matter: the 16 DMA
    engines are latency-bound ~376ns/descriptor below ~10KB."""
    assert kc >= 5
    plan = [("0", 0)]
    ck = 2
    while kc - ck >= 5:
        plan.append(("p", ck))
        ck += 2
    if kc - ck == 3:
        plan.append(("t", ck))
    else:
        plan.append(("p", ck))
        plan.append(("p", ck + 2))
    return plan


def build_nc(kc: int) -> bass.Bass:
    """kc = number of 128-row contraction chunks (last zero-padded on host).

    All chunk tiles are resident in SBUF (no buffer reuse), and all input
    DMAs are issued upfront so the 16 DMA engines never starve. The fp16
    lhs bytes ride appended to pair 0's lines (a standalone lhs transfer
    has 1664B lines and burns ~3us of latency-bound descriptor time at
    the queue head); matmuls read them through a bitcast AP."""
    nc = bass.Bass()
    plan = _xfer_plan(kc)
    n_pair = sum(1 for t, _ in plan if t == "p")
    LB = kc * B * 2          # lhs bytes per partition line
    LOFF = 2 * ESH           # lhs byte offset within pair-0 lines

    n_trip = sum(1 for t, _ in plan if t == "t")
    Bmp0_d = nc.dram_tensor("Bmp0", [128, 2 * ESH + LB], f8,
                            kind="ExternalInput")
    if n_pair:
        Bmp_d = nc.dram_tensor("Bmp", [n_pair, 128, 2 * ESH], f8,
                               kind="ExternalInput")
    if n_trip:
        Bmt_d = nc.dram_tensor("Bmt", [128, 3 * ESH], f8, kind="ExternalInput")
    # out[half, b, x] = sigmoid score for entity half*HALF+x -> 5000B dram
    # lines per (half, b), so the stores use big contiguous descriptors
    out_d = nc.dram_tensor("out", [2, B, ESH // 2], f16, kind="ExternalOutput")

    HALF = ESH // 2   # 2500
    NS = ESH // SUBW  # 10 sub-accumulators; 2 per PSUM bank (split partitions)

    with tile.TileContext(nc) as tc:
        with (
            tc.tile_pool(name="const", bufs=1) as cpool,
            tc.tile_pool(name="ps", bufs=1, space=bass.MemorySpace.PSUM) as pspool,
            tc.tile_pool(name="acc", bufs=1) as accpool,
        ):
            warm = cpool.tile([128, 1], f32, tag="warm")
            out2 = accpool.tile([128, HALF], f16, tag="outsb")

            # load the Sigmoid act table early (it's the only table this
            # kernel uses, so the tail pays no table switch)
            nc.scalar.activation(warm[:], warm[:], AF.Sigmoid)

            # all chunk DMAs upfront on gpsimd = SWDGE (descriptors spread
            # across all 16 DMA engines)
            chunk_src = {}   # ck -> (tile, col_off)
            pi = 0
            p0 = None
            for t, ck in plan:
                if t == "0":
                    p0 = cpool.tile([128, 2 * ESH + LB], f8, tag="btp0")
                    nc.gpsimd.dma_start(p0[:], Bmp0_d[:])
                    chunk_src[ck] = (p0, 0)
                    chunk_src[ck + 1] = (p0, ESH)
                elif t == "p":
                    tl = cpool.tile([128, 2 * ESH], f8, tag=f"btp{pi + 1}")
                    nc.gpsimd.dma_start(tl[:], Bmp_d[pi, :, :])
                    chunk_src[ck] = (tl, 0)
                    chunk_src[ck + 1] = (tl, ESH)
                    pi += 1
                else:
                    tl = cpool.tile([128, 3 * ESH], f8, tag="btt")
                    nc.gpsimd.dma_start(tl[:], Bmt_d[:])
                    chunk_src[ck] = (tl, 0)
                    chunk_src[ck + 1] = (tl, ESH)
                    chunk_src[ck + 2] = (tl, 2 * ESH)

            ps = [
                pspool.tile([128, SUBW], f32, tag=f"ps{s}", name=f"ps_{s}")
                for s in range(NS // 2)
            ]

            def acc_mm(ck, s, start, stop):
                t, off = chunk_src[ck]
                bank, half = s % 5, s // 5
                nc.tensor.matmul(
                    ps[bank][half * B : (half + 1) * B, :],
                    p0[:, LOFF + ck * 2 * B : LOFF + (ck + 1) * 2 * B].bitcast(f16),
                    t[:, off + s * SUBW : off + (s + 1) * SUBW],
                    start=start, stop=stop, tile_position=(0, half * B),
                )

            # chunks 0..kc-3: plain order
            for ck in range(kc - 2):
                for s in range(NS):
                    acc_mm(ck, s, start=(ck == 0), stop=False)
            # final two chunks interleaved bank-major so each PSUM bank's
            # accumulation STOPS early and its sigmoid pipelines under the
            # remaining matmuls. Banks 4,3 stop FIRST so their store (the
            # one that would otherwise queue behind the other store's
            # engine time) launches while banks 0-2 still accumulate.
            for s in (4, 9, 3, 8, 2, 7, 1, 6, 0, 5):
                acc_mm(kc - 2, s, start=False, stop=False)
                acc_mm(kc - 1, s, start=False, stop=True)

            # tail: per PSUM bank, sigmoid straight from PSUM into fp16
            # (rows 0:64 = entity half [0,2500), rows 64:128 = [2500,5000)),
            # in bank-stop order 4,3,2,1,0 matching the interleave above:
            # banks 4,3's store fires mid-matmul-tail, banks 0-2's store
            # follows the last sigmoid without queueing behind it
            for bank in (4, 3, 2, 1, 0):
                c0 = bank * SUBW
                nc.scalar.activation(
                    out2[:, c0 : c0 + SUBW], ps[bank][:], AF.Sigmoid,
                )
                if bank == 3:
                    nc.gpsimd.dma_start(
                        out_d[:, :, 3 * SUBW : HALF], out2[:, 3 * SUBW : HALF]
                    )
            nc.gpsimd.dma_start(
                out_d[:, :, 0 : 3 * SUBW], out2[:, 0 : 3 * SUBW]
            )

    _split_multi_waits(nc)
    return nc


def _bspline3(t):
    at = np.abs(t)
    r = np.zeros_like(at)
    m1 = at < 1
    r[m1] = (4 - 6 * at[m1] ** 2 + 3 * at[m1] ** 3) / 6
    m2 = (at >= 1) & (at < 2)
    r[m2] = ((2 - at[m2]) ** 3) / 6
    return r


def _rtn8(x):
    """Round f64 -> fp8e3 representable, returned as f64 (same scale)."""
    return np.asarray(x, dtype=np.float32).astype(FP8).astype(np.float64)


def _quant_ed3(vals, M4):
    """vals [4, E]: S8-scaled spline window values. M4 [E, 4, 4]: covariance
    of the w-weighted spline coefficients over each window. Picks, per
    entity, the floor/ceil combo of the 4 taps minimizing eps^T M4 eps
    (the expected squared error of the weighted sum over the batch)."""
    near = np.empty_like(vals)
    alt = np.empty_like(vals)
    for j in range(4):
        v = vals[j]
        q = _rtn8(v)
        step = np.maximum(np.abs(q) * 2.0 ** -4, 2.0 ** -6)
        alt[j] = _rtn8(np.where(q > v, q - step, q + step))
        near[j] = q
    best_cost = np.full(vals.shape[1], np.inf)
    best = np.empty_like(vals)
    for m in range(16):
        bits = np.array([(m >> j) & 1 for j in range(4)])[:, None]
        pick = np.where(bits == 1, alt, near)
        eps = pick - vals
        cost = np.einsum('ae,eab,be->e', eps, M4, eps)
        sel = cost < best_cost
        best_cost = np.where(sel, cost, best_cost)
        best = np.where(sel[None, :], pick, best)
    return best


def make_host_data(e1_idx, r_idx, E_weight, R_weight, num_lit, c, var, nf_weights):
    """Build the fp8 basis matrix [K, NUM_ENT] and packed fp16 lhs."""
    e1_idx = np.asarray(e1_idx).astype(np.int64)
    r_idx = np.asarray(r_idx).astype(np.int64)
    E_weight = np.asarray(E_weight, dtype=np.float64)
    R_weight = np.asarray(R_weight, dtype=np.float64)
    num_lit = np.asarray(num_lit, dtype=np.float64)
    c = np.asarray(c, dtype=np.float64)
    var = np.asarray(var, dtype=np.float64)
    nf = np.asarray(nf_weights, dtype=np.float64)

    sig = np.sqrt(var)                  # [L]
    a_ctr = num_lit[e1_idx] - c         # [B, L] Gaussian centers, z units
    w = nf[r_idx]                       # [B, L]

    Js = np.array([int(np.ceil(1.0 / (H * s))) + GUARD for s in sig])
    offs = np.concatenate([[0], np.cumsum(Js)[:-1]])
    K_phi = int(Js.sum())
    K = K_phi + DIM
    kc = (K + 127) // 128

    Bmat = np.zeros((K, NUM_ENT), dtype=np.float64)
    lhsT = np.zeros((K, B), dtype=np.float16)
    eidx = np.arange(NUM_ENT)
    sidx = np.arange(0, NUM_ENT, NUM_ENT // 4000)   # refit subsample
    for l in range(N_LIT):
        hz = H * sig[l]
        J = int(Js[l])
        off = int(offs[l])
        xi0 = -2 * hz
        t = (num_lit[:, l] - xi0) / hz
        j0 = np.clip(np.floor(t).astype(np.int64), 1, J - 3)
        # 4-tap window of B-spline values, quantized to fp8 (at scale S8)
        # with covariance-weighted rounding: per entity, pick the floor/
        # ceil combo of the 4 taps that minimizes the expected squared
        # error of the coefficient-weighted sum over the batch.
        zfit = np.linspace(-2 * hz, 1 + 2 * hz, 4 * J)
        xi = xi0 + hz * np.arange(J)
        Bz = _bspline3((zfit[:, None] - xi[None, :]) / hz)
        G0 = Bz.T @ Bz + 1e-9 * np.eye(J)
        S0 = np.exp(-(((a_ctr[:, l][:, None] - zfit[None, :]) / sig[l]) ** 2))
        C0 = np.linalg.solve(G0, Bz.T @ S0.T).T * w[:, l][:, None]
        M = C0.T @ C0 / B + 1e-12 * np.eye(J)
        jw = j0[None, :] - 1 + np.arange(4)[:, None]      # [4, E]
        M4 = np.moveaxis(M[jw[:, None, :], jw[None, :, :]], 2, 0)
        win = np.stack([_bspline3(t - (j0 - 1 + k)) for k in range(4)]) * S8
        winq = _quant_ed3(win, M4)
        for k in range(4):
            Bmat[off + (j0 - 1 + k), eidx] = winq[k]
        # refit: per-batch LSQ of the true Gaussian against the *quantized*
        # basis at a subsample of actual entity literal values (this also
        # absorbs the 1/S8 scale into the coefficients)
        Bs = Bmat[off : off + J][:, sidx]
        G = Bs @ Bs.T + 1e-8 * np.eye(J)
        phi = np.exp(
            -(((a_ctr[:, l][:, None] - num_lit[sidx, l][None, :]) / sig[l]) ** 2)
        )
        C = np.linalg.solve(G, Bs @ phi.T).T            # [B, J]
        lhsT[off : off + J, :] = (C * w[:, l][:, None]).T.astype(np.float16)

    # append the DistMult rows: score_l = (e1*r) @ E^T
    x = E_weight[e1_idx] * R_weight[r_idx]              # [B, D]
    Bmat[K_phi : K_phi + DIM, :] = _rtn8(E_weight.T * S8)
    lhsT[K_phi : K_phi + DIM, :] = (x.T / S8).astype(np.float16)

    Bmat8 = Bmat.astype(np.float32).astype(FP8)         # exact (pre-rounded)

    # pack lhs chunks (zero-pad K -> kc*128): lhs_pack[p, ck*B+b] = lhsT[ck*128+p, b]
    lhs_pad = np.zeros((kc * 128, B), dtype=np.float16)
    lhs_pad[:K] = lhsT
    lhs_pack = np.ascontiguousarray(
        lhs_pad.reshape(kc, 128, B).transpose(1, 0, 2).reshape(128, kc * B)
    )
    return kc, Bmat8, lhs_pack


def make_in_maps_from(kc, Bmat8, lhs_pack):
    plan = _xfer_plan(kc)
    n_pair = sum(1 for t, _ in plan if t == "p")
    K = Bmat8.shape[0]
    lhs_bytes = lhs_pack.view(FP8)           # [128, kc*B*2] raw fp16 bytes
    in_maps = []
    for core in range(NCORES):
        sl = slice(core * ESH, (core + 1) * ESH)
        Bpad = np.zeros((kc * 128, ESH), dtype=FP8)
        Bpad[:K] = Bmat8[:, sl]
        ch = lambda ck: Bpad[ck * 128 : (ck + 1) * 128]
        pairs = np.empty((n_pair, 128, 2 * ESH), dtype=FP8)
        m = {}
        pi = 0
        for t, ck in plan:
            if t == "0":
                m["Bmp0"] = np.ascontiguousarray(
                    np.concatenate([ch(ck), ch(ck + 1), lhs_bytes], axis=1)
                )
            elif t == "p":
                pairs[pi, :, :ESH] = ch(ck)
                pairs[pi, :, ESH:] = ch(ck + 1)
                pi += 1
            else:
                m["Bmt"] = np.ascontiguousarray(
                    np.concatenate([ch(ck), ch(ck + 1), ch(ck + 2)], axis=1)
                )
        if n_pair:
            m["Bmp"] = pairs
        in_maps.append(m)
    return in_maps


def make_in_maps(**inputs):
    kc, Bmat8, lhs_pack = make_host_data(**inputs)
    return make_in_maps_from(kc, Bmat8, lhs_pack)


_NC_CACHE = {}


def kernel(**inputs) -> np.ndarray:
    kc, Bmat8, lhs_pack = make_host_data(**inputs)
    if kc not in _NC_CACHE:
        _NC_CACHE[kc] = build_nc(kc)
    nc = _NC_CACHE[kc]
    in_maps = make_in_maps_from(kc, Bmat8, lhs_pack)
    res = run_bass_kernel_spmd(nc, in_maps, list(range(NCORES)))
    # per-core out is [2, B, ESH//2] (entity halves); reassemble to [B, ESH]
    out = np.concatenate(
        [
            np.concatenate(
                [np.asarray(res.results[i]["out"][h]) for h in range(2)], axis=1
            )
            for i in range(NCORES)
        ],
        axis=1,
    )
    return out.astype(np.float32)


# revision 28
# speedup vs baseline: 1.2375x; 1.0999x over previous
"""DistMult+KBLN scoring kernel for 8 Trainium2 NeuronCores.

Math (eval mode, per reference):
    e1 = E[e1_idx]; r = R[r_idx]                       [B, D]
    score_l[b,e] = sum_d (e1*r)[b,d] * E[e,d]
    score_n[b,e] = sum_l nf[r_idx][b,l] * exp(-((n_h[b,l]-num_lit[e,l]-c[l])^2/var[l]))
    out = sigmoid(score_l + score_n)                   [B, E]

Key idea: the RBF factor phi(a - m) is a smooth 1-D Gaussian in the
num_lit value m, so expand it per-literal in a cubic B-spline basis on a
knot grid over m (spacing H*sigma_l):

    phi(a[b,l] - m[e,l]) ~= sum_j c_j(a[b,l]) * B3((m[e,l]-xi_j)/h_l)

Then score_n[b,e] = sum_{l,j} (w[b,l]*c_{l,j}) * Bmat[(l,j), e] is ONE
matmul with contraction dim K ~= 1650 (score_l's 200 E-rows are appended
to the same contraction).  The basis matrix is stored in FP8 (E3M4) with
error-diffusion rounding along the 4-tap spline window (the taps' errors
cancel against the smooth spline coefficients), and the per-batch
coefficients are least-squares refit against the *quantized* basis
sampled at actual entity literal values.  The stationary lhs stays FP16
(the PE supports fp16-stationary x fp8-moving matmuls).  The device
kernel is a pure streaming matmul: ~8 MB of fp8 basis per core streamed
from HBM at full DMA bandwidth, accumulated in PSUM (10 sub-accumulators
as halves of 5 banks), sigmoid on the way out, fp16 store.

Sharding: entity axis split row-wise across 8 cores (5000 entities
each); host concatenates. No collectives.
"""
import sys

if "/opt/trn_rl_repo" not in sys.path:
    sys.path.insert(0, "/opt/trn_rl_repo")

import ml_dtypes
import numpy as np

import concourse.bass as bass
import concourse.mybir as mybir
import concourse.tile as _tile
from concourse import tile
from concourse.bass_utils import run_bass_kernel_spmd
from concourse.vector_clock import ScopedClock

B = 64
NUM_ENT = 40000
NUM_REL = 1345
DIM = 200
N_LIT = 100
NCORES = 8
ESH = NUM_ENT // NCORES  # 5000 entities per core

SUBW = 500   # matmul free dim (one PSUM bank holds 512 fp32)
H = 0.69     # B-spline knot spacing in units of sigma_l
GUARD = 4    # extra knots beyond 1/h (cubic overhang)
S8 = 16.0    # fp8 scale: basis values stored as fp8(S8*B3), lhs carries 1/S8
FP8 = ml_dtypes.float8_e3m4

f32 = mybir.dt.float32
f16 = mybir.dt.float16
f8 = mybir.dt.float8e3
AF = mybir.ActivationFunctionType


def _drain_and_barrier_split(self, tick_clock, wait_clock):
    # This walrus build rejects >1 sync-wait per instruction; the tail Drain
    # normally carries one wait per active processor. Collect them on a probe
    # NOP instead (split later by _split_multi_waits) and emit a clean drain.
    nc = self.nc
    probe = nc.sync.nop(nofuse=True, hint="tail_wait_probe")
    wait_clock.add_sem_waits(probe.ins, ScopedClock({None: tick_clock.global_clock}))
    nc.sync.drain()
    nc.all_engine_barrier()
    assert self.sems is not None
    popped = nc._tile_sem_poison_stack.pop()
    assert popped is self._sem_poison
    nc.clear_and_free_semaphores(list(self.sems.allocated().values()))
    nc.all_engine_barrier()


_tile.TileContext._drain_and_barrier = _drain_and_barrier_split


def _split_multi_waits(nc: bass.Bass) -> int:
    """Hoist all-but-one sync wait from every instruction onto standalone
    single-wait EventSemaphore instructions inserted just before it (same
    engine, same block). Needed because this walrus build errors with
    "Too many sync wait commands" on instructions carrying >1 wait."""
    n_split = 0
    for bb in nc.m.functions[0].blocks:
        new_insts = []
        for inst in bb.instructions:
            waits = list(inst.sync_info.on_wait) if inst.sync_info else []
            if len(waits) > 1:
                for sw in waits[:-1]:
                    ev = mybir.InstEventSemaphore(
                        name=nc.get_next_instruction_name(),
                        engine=inst.engine,
                        ins=[],
                        outs=[],
                        sync_info=mybir.SyncInfo(on_wait=[sw], on_update=[]),
                    )
                    nc.register_instruction(ev)
                    new_insts.append(ev)
                    n_split += 1
                inst.sync_info.on_wait = waits[-1:]
            new_insts.append(inst)
        bb.instructions[:] = new_insts
    return n_split


def _xfer_plan(kc: int):
    """DMA transfer plan over 128-row chunks (the last one zero-padded to
    128 rows on the host): pair 0 carries the fp16 lhs bytes appended to
    its lines ('0'), the final three chunks ride one triple-wide transfer
    ('t'), the rest go in pairs ('p'). Wide lines matter: the 16 DMA
    engines are latency-bound ~376ns/descriptor below ~10KB."""
    assert kc >= 5
    plan = [("0", 0)]
    ck = 2
    while kc - ck >= 5:
        plan.append(("p", ck))
        ck += 2
    if kc - ck == 3:
        plan.append(("t", ck))
    else:
        plan.append(("p", ck))
        plan.append(("p", ck + 2))
    return plan


def build_nc(kc: int) -> bass.Bass:
    """kc = number of 128-row contraction chunks (last zero-padded on host).

    All chunk tiles are resident in SBUF (no buffer reuse), and all input
    DMAs are issued upfront so the 16 DMA engines never starve. The fp16
    lhs bytes ride appended to pair 0's lines (a standalone lhs transfer
    has 1664B lines and burns ~3us of latency-bound descriptor time at
    the queue head); matmuls read them through a bitcast AP."""
    nc = bass.Bass()
    plan = _xfer_plan(kc)
    n_pair = sum(1 for t, _ in plan if t == "p")
    LB = kc * B * 2          # lhs bytes per partition line
    LOFF = 2 * ESH           # lhs byte offset within pair-0 lines

    n_trip = sum(1 for t, _ in plan if t == "t")
    Bmp0_d = nc.dram_tensor("Bmp0", [128, 2 * ESH + LB], f8,
                            kind="ExternalInput")
    if n_pair:
        Bmp_d = nc.dram_tensor("Bmp", [n_pair, 128, 2 * ESH], f8,
                               kind="ExternalInput")
    if n_trip:
        Bmt_d = nc.dram_tensor("Bmt", [128, 3 * ESH], f8, kind="ExternalInput")
    # out[half, b, x] = sigmoid score for entity half*HALF+x -> 5000B dram
    # lines per (half, b), so the stores use big contiguous descriptors
    out_d = nc.dram_tensor("out", [2, B, ESH // 2], f16, kind="ExternalOutput")

    HALF = ESH // 2   # 2500
    NS = ESH // SUBW  # 10 sub-accumulators; 2 per PSUM bank (split partitions)

    with tile.TileContext(nc) as tc:
        with (
            tc.tile_pool(name="const", bufs=1) as cpool,
            tc.tile_pool(name="ps", bufs=1, space=bass.MemorySpace.PSUM) as pspool,
            tc.tile_pool(name="acc", bufs=1) as accpool,
        ):
            warm = cpool.tile([128, 1], f32, tag="warm")
            out2 = accpool.tile([128, HALF], f16, tag="outsb")

            # load the Sigmoid act table early (it's the only table this
            # kernel uses, so the tail pays no table switch)
            nc.scalar.activation(warm[:], warm[:], AF.Sigmoid)

            # all chunk DMAs upfront on gpsimd = SWDGE (descriptors spread
            # across all 16 DMA engines)
            chunk_src = {}   # ck -> (tile, col_off)
            pi = 0
            p0 = None
            for t, ck in plan:
                if t == "0":
                    p0 = cpool.tile([128, 2 * ESH + LB], f8, tag="btp0")
                    nc.gpsimd.dma_start(p0[:], Bmp0_d[:])
                    chunk_src[ck] = (p0, 0)
                    chunk_src[ck + 1] = (p0, ESH)
                elif t == "p":
                    tl = cpool.tile([128, 2 * ESH], f8, tag=f"btp{pi + 1}")
                    nc.gpsimd.dma_start(tl[:], Bmp_d[pi, :, :])
                    chunk_src[ck] = (tl, 0)
                    chunk_src[ck + 1] = (tl, ESH)
                    pi += 1
                else:
                    tl = cpool.tile([128, 3 * ESH], f8, tag="btt")
                    nc.gpsimd.dma_start(tl[:], Bmt_d[:])
                    chunk_src[ck] = (tl, 0)
                    chunk_src[ck + 1] = (tl, ESH)
                    chunk_src[ck + 2] = (tl, 2 * ESH)

            ps = [
                pspool.tile([128, SUBW], f32, tag=f"ps{s}", name=f"ps_{s}")
                for s in range(NS // 2)
            ]

            def acc_mm(ck, s, start, stop):
                t, off = chunk_src[ck]
                bank, half = s % 5, s // 5
                nc.tensor.matmul(
                    ps[bank][half * B : (half + 1) * B, :],
                    p0[:, LOFF + ck * 2 * B : LOFF + (ck + 1) * 2 * B].bitcast(f16),
                    t[:, off + s * SUBW : off + (s + 1) * SUBW],
                    start=start, stop=stop, tile_position=(0, half * B),
                )

            # chunks 0..kc-4: plain order
            for ck in range(kc - 3):
                for s in range(NS):
                    acc_mm(ck, s, start=(ck == 0), stop=False)
            # final three chunks interleaved bank-major so each PSUM bank's
            # accumulation STOPS early and its sigmoid + store pipeline
            # under the remaining ~6us of matmuls. Banks 4,3 stop FIRST so
            # their store's ~3us of descriptor-engine time fully drains
            # before the banks-0-2 store becomes ready.
            for s in (4, 9, 3, 8, 2, 7, 1, 6, 0, 5):
                acc_mm(kc - 3, s, start=False, stop=False)
                acc_mm(kc - 2, s, start=False, stop=False)
                acc_mm(kc - 1, s, start=False, stop=True)

            # tail: per PSUM bank, sigmoid straight from PSUM into fp16
            # (rows 0:64 = entity half [0,2500), rows 64:128 = [2500,5000)),
            # in bank-stop order 4,3,2,1,0 matching the interleave above:
            # banks 4,3's store fires mid-matmul-tail, banks 0-2's store
            # follows the last sigmoid without queueing behind it
            for bank in (4, 3, 2, 1, 0):
                c0 = bank * SUBW
                nc.scalar.activation(
                    out2[:, c0 : c0 + SUBW], ps[bank][:], AF.Sigmoid,
                )
                if bank == 3:
                    nc.gpsimd.dma_start(
                        out_d[:, :, 3 * SUBW : HALF], out2[:, 3 * SUBW : HALF]
                    )
            nc.gpsimd.dma_start(
                out_d[:, :, 0 : 3 * SUBW], out2[:, 0 : 3 * SUBW]
            )

    _split_multi_waits(nc)
    return nc


def _bspline3(t):
    at = np.abs(t)
    r = np.zeros_like(at)
    m1 = at < 1
    r[m1] = (4 - 6 * at[m1] ** 2 + 3 * at[m1] ** 3) / 6
    m2 = (at >= 1) & (at < 2)
    r[m2] = ((2 - at[m2]) ** 3) / 6
    return r


def _rtn8(x):
    """Round f64 -> fp8e3 representable, returned as f64 (same scale)."""
    return np.asarray(x, dtype=np.float32).astype(FP8).astype(np.float64)


def _quant_ed3(vals, M4):
    """vals [4, E]: S8-scaled spline window values. M4 [E, 4, 4]: covariance
    of the w-weighted spline coefficients over each window. Picks, per
    entity, the floor/ceil combo of the 4 taps minimizing eps^T M4 eps
    (the expected squared error of the weighted sum over the batch)."""
    near = np.empty_like(vals)
    alt = np.empty_like(vals)
    for j in range(4):
        v = vals[j]
        q = _rtn8(v)
        step = np.maximum(np.abs(q) * 2.0 ** -4, 2.0 ** -6)
        alt[j] = _rtn8(np.where(q > v, q - step, q + step))
        near[j] = q
    best_cost = np.full(vals.shape[1], np.inf)
    best = np.empty_like(vals)
    for m in range(16):
        bits = np.array([(m >> j) & 1 for j in range(4)])[:, None]
        pick = np.where(bits == 1, alt, near)
        eps = pick - vals
        cost = np.einsum('ae,eab,be->e', eps, M4, eps)
        sel = cost < best_cost
        best_cost = np.where(sel, cost, best_cost)
        best = np.where(sel[None, :], pick, best)
    return best


def make_host_data(e1_idx, r_idx, E_weight, R_weight, num_lit, c, var, nf_weights):
    """Build the fp8 basis matrix [K, NUM_ENT] and packed fp16 lhs."""
    e1_idx = np.asarray(e1_idx).astype(np.int64)
    r_idx = np.asarray(r_idx).astype(np.int64)
    E_weight = np.asarray(E_weight, dtype=np.float64)
    R_weight = np.asarray(R_weight, dtype=np.float64)
    num_lit = np.asarray(num_lit, dtype=np.float64)
    c = np.asarray(c, dtype=np.float64)
    var = np.asarray(var, dtype=np.float64)
    nf = np.asarray(nf_weights, dtype=np.float64)

    sig = np.sqrt(var)                  # [L]
    a_ctr = num_lit[e1_idx] - c         # [B, L] Gaussian centers, z units
    w = nf[r_idx]                       # [B, L]

    Js = np.array([int(np.ceil(1.0 / (H * s))) + GUARD for s in sig])
    offs = np.concatenate([[0], np.cumsum(Js)[:-1]])
    K_phi = int(Js.sum())
    K = K_phi + DIM
    kc = (K + 127) // 128

    Bmat = np.zeros((K, NUM_ENT), dtype=np.float64)
    lhsT = np.zeros((K, B), dtype=np.float16)
    eidx = np.arange(NUM_ENT)
    sidx = np.arange(0, NUM_ENT, NUM_ENT // 4000)   # refit subsample
    for l in range(N_LIT):
        hz = H * sig[l]
        J = int(Js[l])
        off = int(offs[l])
        xi0 = -2 * hz
        t = (num_lit[:, l] - xi0) / hz
        j0 = np.clip(np.floor(t).astype(np.int64), 1, J - 3)
        # 4-tap window of B-spline values, quantized to fp8 (at scale S8)
        # with covariance-weighted rounding: per entity, pick the floor/
        # ceil combo of the 4 taps that minimizes the expected squared
        # error of the coefficient-weighted sum over the batch.
        zfit = np.linspace(-2 * hz, 1 + 2 * hz, 4 * J)
        xi = xi0 + hz * np.arange(J)
        Bz = _bspline3((zfit[:, None] - xi[None, :]) / hz)
        G0 = Bz.T @ Bz + 1e-9 * np.eye(J)
        S0 = np.exp(-(((a_ctr[:, l][:, None] - zfit[None, :]) / sig[l]) ** 2))
        C0 = np.linalg.solve(G0, Bz.T @ S0.T).T * w[:, l][:, None]
        M = C0.T @ C0 / B + 1e-12 * np.eye(J)
        jw = j0[None, :] - 1 + np.arange(4)[:, None]      # [4, E]
        M4 = np.moveaxis(M[jw[:, None, :], jw[None, :, :]], 2, 0)
        win = np.stack([_bspline3(t - (j0 - 1 + k)) for k in range(4)]) * S8
        winq = _quant_ed3(win, M4)
        for k in range(4):
            Bmat[off + (j0 - 1 + k), eidx] = winq[k]
        # refit: per-batch LSQ of the true Gaussian against the *quantized*
        # basis at a subsample of actual entity literal values (this also
        # absorbs the 1/S8 scale into the coefficients)
        Bs = Bmat[off : off + J][:, sidx]
        G = Bs @ Bs.T + 1e-8 * np.eye(J)
        phi = np.exp(
            -(((a_ctr[:, l][:, None] - num_lit[sidx, l][None, :]) / sig[l]) ** 2)
        )
        C = np.linalg.solve(G, Bs @ phi.T).T            # [B, J]
        lhsT[off : off + J, :] = (C * w[:, l][:, None]).T.astype(np.float16)

    # append the DistMult rows: score_l = (e1*r) @ E^T
    x = E_weight[e1_idx] * R_weight[r_idx]              # [B, D]
    Bmat[K_phi : K_phi + DIM, :] = _rtn8(E_weight.T * S8)
    lhsT[K_phi : K_phi + DIM, :] = (x.T / S8).astype(np.float16)

    Bmat8 = Bmat.astype(np.float32).astype(FP8)         # exact (pre-rounded)

    # pack lhs chunks (zero-pad K -> kc*128): lhs_pack[p, ck*B+b] = lhsT[ck*128+p, b]
    lhs_pad = np.zeros((kc * 128, B), dtype=np.float16)
    lhs_pad[:K] = lhsT
    lhs_pack = np.ascontiguousarray(
        lhs_pad.reshape(kc, 128, B).transpose(1, 0, 2).reshape(128, kc * B)
    )
    return kc, Bmat8, lhs_pack


def make_in_maps_from(kc, Bmat8, lhs_pack):
    plan = _xfer_plan(kc)
    n_pair = sum(1 for t, _ in plan if t == "p")
    K = Bmat8.shape[0]
    lhs_bytes = lhs_pack.view(FP8)           # [128, kc*B*2] raw fp16 bytes
    in_maps = []
    for core in range(NCORES):
        sl = slice(core * ESH, (core + 1) * ESH)
        Bpad = np.zeros((kc * 128, ESH), dtype=FP8)
        Bpad[:K] = Bmat8[:, sl]
        ch = lambda ck: Bpad[ck * 128 : (ck + 1) * 128]
        pairs = np.empty((n_pair, 128, 2 * ESH), dtype=FP8)
        m = {}
        pi = 0
        for t, ck in plan:
            if t == "0":
                m["Bmp0"] = np.ascontiguousarray(
                    np.concatenate([ch(ck), ch(ck + 1), lhs_bytes], axis=1)
                )
            elif t == "p":
                pairs[pi, :, :ESH] = ch(ck)
                pairs[pi, :, ESH:] = ch(ck + 1)
                pi += 1
            else:
                m["Bmt"] = np.ascontiguousarray(
                    np.concatenate([ch(ck), ch(ck + 1), ch(ck + 2)], axis=1)
                )
        if n_pair:
            m["Bmp"] = pairs
        in_maps.append(m)
    return in_maps


def make_in_maps(**inputs):
    kc, Bmat8, lhs_pack = make_host_data(**inputs)
    return make_in_maps_from(kc, Bmat8, lhs_pack)


_NC_CACHE = {}


def kernel(**inputs) -> np.ndarray:
    kc, Bmat8, lhs_pack = make_host_data(**inputs)
    if kc not in _NC_CACHE:
        _NC_CACHE[kc] = build_nc(kc)
    nc = _NC_CACHE[kc]
    in_maps = make_in_maps_from(kc, Bmat8, lhs_pack)
    res = run_bass_kernel_spmd(nc, in_maps, list(range(NCORES)))
    # per-core out is [2, B, ESH//2] (entity halves); reassemble to [B, ESH]
    out = np.concatenate(
        [
            np.concatenate(
                [np.asarray(res.results[i]["out"][h]) for h in range(2)], axis=1
            )
            for i in range(NCORES)
        ],
        axis=1,
    )
    return out.astype(np.float32)
